# revision 19
# baseline (speedup 1.0000x reference)
"""Causal multi-head attention block on 8 NeuronCores (Trainium2, Bass/Tile).

Reference computation (per batch b):
  Q = x @ W_Q + b_Q ; K = x @ W_K (+ b_K, dropped: softmax shift-invariant)
  scores = Q K^T / sqrt(H); causal mask; probs = softmax(scores)
  out = (probs @ V) @ W_O + b_O   (b_V folded on host: probs rows sum to 1)

Sharding: core c -> batch c//2, head-group c%2 (6 of 12 heads). Host sums
the two head-group partials per batch and adds b_O + sum_nh b_V[n,h]*W_O.

Numerics (fp8 DoubleRow): weights are host-folded x32 so fp8 residual
splits stay in e4m3's normal range. Projections run as fp8 DoubleRow
matmuls (0.5 cy/row, 2 k-tiles per instr): Q/K/V = xh*Wh + xls*(Wh/16)
[+ (xh/16)*Wls in 3-term mode]. Q is stored to SBUF as an fp8 hi+lo pair
(bias folded into hi via tensor_scalar_add; lo via scalar_tensor_tensor),
K as fp8 hi. Scores: one DR instr per (head, ktile): slots
(Kh, Qh) + (Kh dup via 0-stride, Qlo). exp runs on Activation into bf16
left-packed per-head strips, merged across ktiles sharing a PSUM tile
(6 exp instrs per head). PV is "flipped": stationary = ets [k, q-block]
(full 128x128 array use), moving = V bf16 [k, 64] -> z [q, h] in a single
PSUM bank per q-tile incl. per-head denominator columns (ones moving).
Normalize = one reciprocal + one broadcast multiply per q-tile; z^T via
bf16 PE transposes; out-proj bf16; bf16 output (host converts to f32).
"""

import sys

sys.path.insert(0, "/opt/trn_rl_repo")

from contextlib import ExitStack

import numpy as np
import ml_dtypes

import concourse.bass as bass
import concourse.tile as tile
from concourse import bacc, mybir
from concourse.bass_utils import run_bass_kernel_spmd

B, S, D, N, H = 4, 1024, 768, 12, 64
NHC = 6            # heads per core
NPAIR = NHC // 2   # head pairs (2 heads -> 128 partitions)
HD = NHC * H       # 384 packed head dim per core
P = 128
NDT = D // P       # 6 d-tiles
NST = S // P       # 8 k-tiles
QB = 512           # strip width (q)
FW = 32.0          # host weight fold (power of 2)
F32 = mybir.dt.float32
F8 = mybir.dt.float8e4
BF = mybir.dt.bfloat16
DR = mybir.MatmulPerfMode.DoubleRow
EXP_SCALE = 1.0 / (np.sqrt(float(H)) * FW * FW)

QK_3T = False      # 3-term QK projections (False: 2-term, faster/less exact)

# wall column layout (fp8 weights, priority-ordered for DMA chunking)
def _wall_layout():
    cols = {}
    off = 0
    def put(name, w):
        nonlocal off
        cols[name] = (off, w)
        off += w
    # chunk 0: pair0 Q then K (two priority sub-chunks)
    for t in ("qh", "qhd") + (("qls",) if QK_3T else ()):
        put(t + "0", P)
    for t in ("kh", "khd") + (("kls",) if QK_3T else ()):
        put(t + "0", P)
    c0 = off
    # chunk 1: V (all pairs)
    for t in ("vh", "vhd", "vls"):
        put(t, HD)
    c1 = off
    # chunks 2, 3: pairs 1, 2
    for g in (1, 2):
        for t in ("qh", "kh", "qhd", "khd") + (("qls", "kls") if QK_3T else ()):
            put(t + str(g), P)
    return cols, (0, c0, c1, (c1 + off) // 2, off)

WALL_COLS, WALL_CHUNKS = _wall_layout()
WALL_W = WALL_CHUNKS[-1]

# ets strip segment offsets (left-packed live columns per ktile)
def _segs(strip):
    offs, off = [], 0
    nkt = 4 if strip == 0 else NST
    for kt in range(nkt):
        if strip == 0:
            live = QB - kt * P
        else:
            live = min(S - kt * P, QB)
        offs.append((off, live))
        off += live
    return offs, off

SEG0, LEN0 = _segs(0)
SEG1, LEN1 = _segs(1)
# exp merge groups: lists of (ktile list, psum cols used)
EXPG0 = [[0, 1], [2, 3]]
EXPG1 = [[0, 1], [2, 3], [4, 5], [6, 7]]

_CACHE = {}


def _build():
    nc = bacc.Bacc()
    wall_d = nc.declare_dram_parameter("wall", [D, WALL_W], F8, isOutput=False)
    xh_d = nc.declare_dram_parameter("xh", [D, S], F8, isOutput=False)
    xls_d = nc.declare_dram_parameter("xls", [D, S], F8, isOutput=False)
    xhd_d = nc.declare_dram_parameter("xhd", [D, S], F8, isOutput=False)
    wo_d = nc.declare_dram_parameter("wo", [HD, D], BF, isOutput=False)
    bq_d = nc.declare_dram_parameter("bq", [P, NPAIR], F32, isOutput=False)
    trid_d = nc.declare_dram_parameter("trid", [P, 2 * P], BF, isOutput=False)
    out_d = nc.declare_dram_parameter("out", [S, D], BF, isOutput=True)
    import os
    DBG = os.environ.get("KDBG") == "1"
    if DBG:
        dq_d = nc.declare_dram_parameter("dq", [P, NPAIR * 2 * S], F8, isOutput=True)
        dk_d = nc.declare_dram_parameter("dk", [P, NPAIR * S], F8, isOutput=True)
        dets_d = nc.declare_dram_parameter("dets", [P, NHC * 3328], BF, isOutput=True)
        dva_d = nc.declare_dram_parameter("dva", [P, NST * NHC * H], BF, isOutput=True)

    wall_r = wall_d[:].rearrange("(t p) c -> p t c", p=P)
    xh_r = xh_d[:].rearrange("(t p) s -> p t s", p=P)
    xls_r = xls_d[:].rearrange("(t p) s -> p t s", p=P)
    xhd_r = xhd_d[:].rearrange("(t p) s -> p t s", p=P)
    wo_r = wo_d[:].rearrange("(t p) d -> p t d", p=P)

    with tile.TileContext(nc) as tc, ExitStack() as ctx:
        consts = ctx.enter_context(tc.tile_pool(name="consts", bufs=1))
        persist = ctx.enter_context(tc.tile_pool(name="persist", bufs=1))
        znp = ctx.enter_context(tc.tile_pool(name="znp", bufs=2))
        zntp = ctx.enter_context(tc.tile_pool(name="zntp", bufs=2))
        outp = ctx.enter_context(tc.tile_pool(name="outp", bufs=2))
        smalls = ctx.enter_context(tc.tile_pool(name="smalls", bufs=2))

        wall = consts.tile([P, NDT, WALL_W], F8)
        xh = consts.tile([P, NDT, S], F8)
        xls = consts.tile([P, NDT, S], F8)
        xhd = consts.tile([P, NDT, S], F8, name="xhd")
        wo = consts.tile([P, NPAIR, D], BF)
        bq = consts.tile([P, NPAIR], F32)
        trid = consts.tile([P, 2 * P], BF)
        tri = trid[:, 0:P]
        ident = trid[:, P : 2 * P]

        def wslice(name, g, t0, nt, c0, cw):
            """stationary slice [P, nt, cw] of wall tensor `name` (pair g)."""
            base = WALL_COLS[name + str(g) if name[0] in "qk" else name][0]
            return wall[:, t0 : t0 + nt, base + c0 : base + c0 + cw]

        # ---- DMA priority order (all on SP/sync queue) ----
        def dma_wall_chunk(i):
            c0, c1 = WALL_CHUNKS[i], WALL_CHUNKS[i + 1]
            nc.sync.dma_start(out=wall[:, :, c0:c1], in_=wall_r[:, :, c0:c1])

        def dma_x_half(t, sb_t, dram_r, s2):
            nc.sync.dma_start(
                out=sb_t[:, :, s2 * QB : (s2 + 1) * QB],
                in_=dram_r[:, :, s2 * QB : (s2 + 1) * QB],
            )

        dma_wall_chunk(0)
        nc.sync.dma_start(out=bq, in_=bq_d[:])
        dma_x_half(0, xh, xh_r, 0)
        nc.sync.dma_start(out=trid, in_=trid_d[:])
        dma_x_half(0, xls, xls_r, 0)
        dma_x_half(0, xhd, xhd_r, 0)
        dma_wall_chunk(1)      # V weights
        dma_wall_chunk(2)      # pair 1
        dma_wall_chunk(3)      # pair 2
        dma_x_half(0, xh, xh_r, 1)
        dma_x_half(0, xls, xls_r, 1)
        dma_x_half(0, xhd, xhd_r, 1)
        nc.sync.dma_start(out=wo, in_=wo_r)

        # ---- persistent activations ----
        qT8 = persist.tile([P, NPAIR, 2, S], F8)   # dim2: hi/lo
        kT8 = persist.tile([P, NPAIR, S], F8)
        vA = persist.tile([P, NST, NHC, H], BF)
        ets0 = persist.tile([P, NHC, LEN0], BF)
        ets1 = persist.tile([P, NHC, LEN1], BF)
        ones = persist.tile([P, 1], BF)
        nc.gpsimd.memset(ones, 1.0)

        ps_sm = ctx.enter_context(tc.tile_pool(name="ps_sm", bufs=2, space="PSUM"))
        ps_z = ctx.enter_context(tc.tile_pool(name="ps_z", bufs=2, space="PSUM"))
        ps_m = ctx.enter_context(tc.tile_pool(name="ps_m", bufs=2, space="PSUM"))

        # PE warm-up: carries the p-state ramp during the DMA prologue.
        dums = consts.tile([P, 2, QB], F8)
        nc.gpsimd.memset(dums, 0.0)
        # preload the Exp activation table off the critical path
        scr = smalls.tile([P, 1], BF, name="scr")
        nc.scalar.activation(scr, ones, mybir.ActivationFunctionType.Exp)
        wps = ps_m.tile([P, QB], F32, name="warm", tag="m")
        for i in range(10):
            nc.tensor.matmul(
                wps, dums[:, :, 0:P], dums, start=(i == 0), stop=(i == 9),
                perf_mode=DR,
            )

        def proj_qk(g, s2):
            """Q and K projections for pair g, s-half s2 (fp8 DR)."""
            s0 = s2 * QB
            k_on_act = (g == 0 and s2 == 0)
            hh_only = (g == 0 and s2 == 0)
            for wname, ap_hi_lo in (("q", True), ("k", False)):
                pp = ps_m.tile([P, QB], F32, tag="m")
                ktp = [(t, t + 2) for t in (0, 2, 4)]
                for i, (t, _) in enumerate(ktp):
                    nc.tensor.matmul(
                        pp, wslice(wname + "h", g, t, 2, 0, P),
                        xh[:, t : t + 2, s0 : s0 + QB],
                        start=(t == 0), stop=(hh_only and i == 2), perf_mode=DR)
                if not hh_only:
                    for i, (t, _) in enumerate(ktp):
                        nc.tensor.matmul(
                            pp, wslice(wname + "hd", g, t, 2, 0, P),
                            xls[:, t : t + 2, s0 : s0 + QB],
                            start=False, stop=(not QK_3T and i == 2),
                            perf_mode=DR)
                    if QK_3T:
                        for i, (t, _) in enumerate(ktp):
                            nc.tensor.matmul(
                                pp, wslice(wname + "ls", g, t, 2, 0, P),
                                xhd[:, t : t + 2, s0 : s0 + QB],
                                start=False, stop=(i == 2), perf_mode=DR)
                if ap_hi_lo:
                    nc.vector.tensor_scalar_add(
                        qT8[:, g, 0, s0 : s0 + QB], pp, bq[:, g : g + 1])
                    nc.vector.scalar_tensor_tensor(
                        out=qT8[:, g, 1, s0 : s0 + QB], in0=pp,
                        scalar=bq[:, g : g + 1], in1=qT8[:, g, 0, s0 : s0 + QB],
                        op0=mybir.AluOpType.add, op1=mybir.AluOpType.subtract)
                elif k_on_act:
                    nc.scalar.copy(kT8[:, g, s0 : s0 + QB], pp)
                else:
                    nc.vector.tensor_copy(out=kT8[:, g, s0 : s0 + QB], in_=pp)

        def proj_v(st):
            """V projection for s-tile st -> vA bf16 (fp8 DR, 3-term)."""
            pp = ps_m.tile([P, HD], F32, tag="m")
            for t in (0, 2, 4):
                nc.tensor.matmul(
                    pp, xh[:, t : t + 2, st * P : (st + 1) * P],
                    wslice("vh", 0, t, 2, 0, HD),
                    start=(t == 0), stop=False, perf_mode=DR)
            for i, t in enumerate((0, 2, 4)):
                nc.tensor.matmul(
                    pp, xls[:, t : t + 2, st * P : (st + 1) * P],
                    wslice("vhd", 0, t, 2, 0, HD),
                    start=False, stop=False, perf_mode=DR)
            for i, t in enumerate((0, 2, 4)):
                nc.tensor.matmul(
                    pp, xhd[:, t : t + 2, st * P : (st + 1) * P],
                    wslice("vls", 0, t, 2, 0, HD),
                    start=False, stop=(i == 2), perf_mode=DR)
            nc.vector.tensor_copy(
                out=vA[:, st, :, :], in_=pp.rearrange("p (n h) -> p n h", n=NHC))

        def scores_head(h, strip):
            """All score DRs + merged exps + masks for head h of strip."""
            g, e = divmod(h, 2)
            hp = e * H
            ets = ets0 if strip == 0 else ets1
            segs = SEG0 if strip == 0 else SEG1
            q0 = strip * QB
            groups = EXPG0 if strip == 0 else EXPG1
            for kts in groups:
                used = sum(segs[kt][1] for kt in kts)
                sm = ps_sm.tile([P, 2 * QB], F32, tag="sm")
                pcol = 0
                for kt in kts:
                    live = segs[kt][1]
                    o = (q0 + QB - live) - q0  # live q starts at q0 + o
                    kst = kT8[hp : hp + H, g, kt * P : (kt + 1) * P]
                    kst = kst.rearrange("p (o m) -> p o m", o=1).broadcast_to((H, 2, P))
                    nc.tensor.matmul(
                        sm[:, pcol : pcol + live], kst,
                        qT8[hp : hp + H, g, :, q0 + o : q0 + QB],
                        start=True, stop=True, perf_mode=DR)
                    pcol += live
                seg0 = segs[kts[0]][0]
                nc.scalar.activation(
                    ets[:, h, seg0 : seg0 + used], sm[:, 0:used],
                    mybir.ActivationFunctionType.Exp, scale=EXP_SCALE)
            # diagonal masks (ets *= tri on first 128 stored cols of diag kts)
            dkts = range(4) if strip == 0 else range(4, 8)
            for kt in dkts:
                so = segs[kt][0]
                nc.gpsimd.tensor_mul(
                    ets[:, h, so : so + P], ets[:, h, so : so + P], tri)

        def pv_qtile(strip, j):
            """z psum for q-tile j of strip: all heads' PV + denominators."""
            ets = ets0 if strip == 0 else ets1
            segs = SEG0 if strip == 0 else SEG1
            q0 = strip * QB
            zp = ps_z.tile([P, 390], F32, tag="z")
            gq = q0 + j * P  # global q block start
            for h in range(NHC):
                kts = [kt for kt in range(len(segs))
                       if kt * P <= gq + P - 1 and True]
                # live ktiles: those whose k-range start <= last q of block
                kts = [kt for kt in range(len(segs)) if kt * P < gq + P]
                for i, kt in enumerate(kts):
                    o = max(kt * P - q0, 0)
                    col = segs[kt][0] + (j * P - o)
                    st = ets[:, h, col : col + P]
                    nc.tensor.matmul(
                        zp[:, h * H : (h + 1) * H], st, vA[:, kt, h, :],
                        start=(i == 0), stop=(i == len(kts) - 1))
                for i, kt in enumerate(kts):
                    o = max(kt * P - q0, 0)
                    col = segs[kt][0] + (j * P - o)
                    st = ets[:, h, col : col + P]
                    nc.tensor.matmul(
                        zp[:, 384 + h : 385 + h], st, ones,
                        start=(i == 0), stop=(i == len(kts) - 1))
            return zp

        def phase3(strip, j, zp, last):
            """normalize -> transpose -> out-proj -> store for q-tile j."""
            row0 = strip * QB + j * P
            r = smalls.tile([P, NHC], F32)
            nc.vector.reciprocal(r, zp[:, 384:390])
            zn = znp.tile([P, NHC, H], BF)
            rb = r.rearrange("p (h o) -> p h o", o=1).broadcast_to((P, NHC, H))
            nc.vector.tensor_mul(
                zn, zp[:, 0:384].rearrange("p (n h) -> p n h", n=NHC), rb)
            znt_ps = ps_m.tile([P, NPAIR, P], BF, tag="m")
            for t in range(NPAIR):
                nc.tensor.matmul(
                    znt_ps[:, t, :], zn[:, 2 * t : 2 * t + 2, :].rearrange(
                        "p n h -> p (n h)"),
                    ident, is_transpose=True)
            znt = zntp.tile([P, NPAIR, P], BF)
            nc.vector.tensor_copy(out=znt, in_=znt_ps)
            osb = outp.tile([P, D], BF)
            for dh in range(2):
                op = ps_m.tile([P, D // 2], F32, tag="m")
                for t in range(NPAIR):
                    nc.tensor.matmul(
                        op, znt[:, t, :],
                        wo[:, t, dh * (D // 2) : (dh + 1) * (D // 2)],
                        start=(t == 0), stop=(t == NPAIR - 1))
                sl = osb[:, dh * (D // 2) : (dh + 1) * (D // 2)]
                if last and dh == 0:
                    nc.scalar.copy(sl, op)
                else:
                    nc.vector.tensor_copy(out=sl, in_=op)
                nc.sync.dma_start(
                    out=out_d[row0 : row0 + P, dh * (D // 2) : (dh + 1) * (D // 2)],
                    in_=sl)

        def phase3_tail(zpA, zpB):
            """Last two q-tiles: interleave DVE/Act chains to shorten the tail."""
            rows = (QB + 2 * P, QB + 3 * P)
            rA = smalls.tile([P, NHC], F32, name="rA")
            rB = smalls.tile([P, NHC], F32, name="rB")
            nc.vector.reciprocal(rA, zpA[:, 384:390])
            nc.vector.reciprocal(rB, zpB[:, 384:390])
            zns, znts = [], []
            for nm, zp, r in (("A", zpA, rA), ("B", zpB, rB)):
                zn = znp.tile([P, NHC, H], BF, name="znt_" + nm)
                rb = r.rearrange("p (h o) -> p h o", o=1).broadcast_to((P, NHC, H))
                nc.vector.tensor_mul(
                    zn, zp[:, 0:384].rearrange("p (n h) -> p n h", n=NHC), rb)
                zns.append(zn)
            for nm, zn in zip("AB", zns):
                znt_ps = ps_m.tile([P, NPAIR, P], BF, tag="m", name="znp_" + nm)
                for t in range(NPAIR):
                    nc.tensor.matmul(
                        znt_ps[:, t, :], zn[:, 2 * t : 2 * t + 2, :].rearrange(
                            "p n h -> p (n h)"),
                        ident, is_transpose=True)
                znt = zntp.tile([P, NPAIR, P], BF, name="zntt_" + nm)
                if nm == "A":
                    nc.scalar.copy(znt, znt_ps)
                else:
                    nc.vector.tensor_copy(out=znt, in_=znt_ps)
                znts.append(znt)
            for dh in range(2):
                for i, znt in enumerate(znts):
                    op = ps_m.tile([P, D // 2], F32, tag="m", name=f"op_{i}{dh}")
                    for t in range(NPAIR):
                        nc.tensor.matmul(
                            op, znt[:, t, :],
                            wo[:, t, dh * (D // 2) : (dh + 1) * (D // 2)],
                            start=(t == 0), stop=(t == NPAIR - 1))
                    osb = outp.tile([P, D // 2], BF, name=f"osb_{i}{dh}")
                    if (i + dh) % 2 == 0:
                        nc.scalar.copy(osb, op)
                    else:
                        nc.vector.tensor_copy(out=osb, in_=op)
                    nc.sync.dma_start(
                        out=out_d[rows[i] : rows[i] + P,
                                  dh * (D // 2) : (dh + 1) * (D // 2)],
                        in_=osb)

        # ================= schedule =================
        proj_qk(0, 0)
        scores_head(0, 0)
        scores_head(1, 0)
        proj_qk(1, 0)
        scores_head(2, 0)
        scores_head(3, 0)
        proj_qk(2, 0)
        scores_head(4, 0)
        scores_head(5, 0)
        proj_qk(0, 1)
        scores_head(0, 1)
        scores_head(1, 1)
        for st in range(4):
            proj_v(st)
        zp0 = pv_qtile(0, 0)
        proj_qk(1, 1)
        scores_head(2, 1)
        scores_head(3, 1)
        zp1 = pv_qtile(0, 1)
        phase3(0, 0, zp0, False)
        proj_qk(2, 1)
        scores_head(4, 1)
        scores_head(5, 1)
        zp2 = pv_qtile(0, 2)
        phase3(0, 1, zp1, False)
        for st in range(4, NST):
            proj_v(st)
        zp3 = pv_qtile(0, 3)
        phase3(0, 2, zp2, False)
        zp4 = pv_qtile(1, 0)
        phase3(0, 3, zp3, False)
        zp5 = pv_qtile(1, 1)
        phase3(1, 0, zp4, True)
        zp6 = pv_qtile(1, 2)
        phase3(1, 1, zp5, True)
        zp7 = pv_qtile(1, 3)
        phase3_tail(zp6, zp7)
        if DBG:
            nc.sync.dma_start(out=dq_d[:], in_=qT8.rearrange("p a b c -> p (a b c)"))
            nc.sync.dma_start(out=dk_d[:], in_=kT8.rearrange("p a b -> p (a b)"))
            nc.sync.dma_start(out=dets_d[:], in_=ets1.rearrange("p a b -> p (a b)"))
            nc.sync.dma_start(out=dva_d[:], in_=vA.rearrange("p a b c -> p (a b c)"))

    if not nc.is_finalized():
        nc.finalize()
    return nc


def _get_program():
    if "nc" not in _CACHE:
        _CACHE["nc"] = _build()
    return _CACHE["nc"]


F8NP = ml_dtypes.float8_e4m3
BFNP = ml_dtypes.bfloat16


def _q8(a):
    return a.astype(F8NP)


def make_in_maps(normalized_resid_pre, W_Q, W_K, W_V, b_Q):
    x = np.asarray(normalized_resid_pre, np.float32)
    W_Q = np.asarray(W_Q, np.float32) * FW
    W_K = np.asarray(W_K, np.float32) * FW
    W_V = np.asarray(W_V, np.float32) * FW
    b_Q = np.asarray(b_Q, np.float32) * FW

    tri = np.triu(np.ones((P, P), np.float32)).astype(BFNP)
    ident = np.eye(P, dtype=np.float32).astype(BFNP)
    trid = np.concatenate([tri, ident], axis=1)

    in_maps = []
    for c in range(8):
        b, hg = divmod(c, 2)
        hs = slice(hg * NHC, (hg + 1) * NHC)
        xt = np.ascontiguousarray(x[b].T)          # [D, S]
        xh = _q8(xt)
        xhf = xh.astype(np.float32)
        xls = _q8((xt - xhf) * 16.0)
        xhd = (xhf / 16.0).astype(F8NP)            # exact exponent shift

        wall = np.zeros((D, WALL_W), F8NP)
        for wname, W in (("q", W_Q), ("k", W_K), ("v", W_V)):
            Wg = W[hs].transpose(1, 0, 2).reshape(D, HD)  # [D, 6*64]
            Wh = _q8(Wg)
            Whf = Wh.astype(np.float32)
            Whd = (Whf / 16.0).astype(F8NP)
            Wls = _q8((Wg - Whf) * 16.0)
            if wname == "v":
                c0 = WALL_COLS["vh"][0]
                wall[:, c0 : c0 + HD] = Wh
                c0 = WALL_COLS["vhd"][0]
                wall[:, c0 : c0 + HD] = Whd
                c0 = WALL_COLS["vls"][0]
                wall[:, c0 : c0 + HD] = Wls
            else:
                for g in range(NPAIR):
                    sl = slice(g * P, (g + 1) * P)
                    c0 = WALL_COLS[wname + "h" + str(g)][0]
                    wall[:, c0 : c0 + P] = Wh[:, sl]
                    c0 = WALL_COLS[wname + "hd" + str(g)][0]
                    wall[:, c0 : c0 + P] = Whd[:, sl]
                    if QK_3T:
                        c0 = WALL_COLS[wname + "ls" + str(g)][0]
                        wall[:, c0 : c0 + P] = Wls[:, sl]

        in_maps.append({
            "wall": wall,
            "xh": xh,
            "xls": xls,
            "xhd": xhd,
            "wo": None,  # filled in kernel()
            "bq": np.ascontiguousarray(b_Q[hs].reshape(NPAIR, P).T),
            "trid": trid,
        })
    return in_maps


def kernel(
    normalized_resid_pre, W_Q, W_K, W_V, W_O, b_Q, b_K, b_V, b_O, **_unused
):
    W_O = np.asarray(W_O, np.float32)
    b_V, b_O = np.asarray(b_V, np.float32), np.asarray(b_O, np.float32)
    in_maps = make_in_maps(normalized_resid_pre, W_Q, W_K, W_V, b_Q)
    for c in range(8):
        hg = c % 2
        hs = slice(hg * NHC, (hg + 1) * NHC)
        in_maps[c]["wo"] = np.ascontiguousarray(
            (W_O[hs].reshape(HD, D) / FW).astype(BFNP))

    nc = _get_program()
    res = run_bass_kernel_spmd(nc, in_maps, list(range(8))).results

    out = np.zeros((B, S, D), np.float32)
    for c in range(8):
        out[c // 2] += res[c]["out"].astype(np.float32)
    out += b_O + np.einsum("nh,nhd->d", b_V, W_O)
    return out


# revision 20
# speedup vs baseline: 1.0046x; 1.0046x over previous
"""Causal multi-head attention block on 8 NeuronCores (Trainium2, Bass/Tile).

Reference computation (per batch b):
  Q = x @ W_Q + b_Q ; K = x @ W_K (+ b_K, dropped: softmax shift-invariant)
  scores = Q K^T / sqrt(H); causal mask; probs = softmax(scores)
  out = (probs @ V) @ W_O + b_O   (b_V folded on host: probs rows sum to 1)

Sharding: core c -> batch c//2, head-group c%2 (6 of 12 heads). Host sums
the two head-group partials per batch and adds b_O + sum_nh b_V[n,h]*W_O.

Numerics (fp8 DoubleRow): weights are host-folded x32 so fp8 residual
splits stay in e4m3's normal range. Projections run as fp8 DoubleRow
matmuls (0.5 cy/row, 2 k-tiles per instr): Q/K/V = xh*Wh + xls*(Wh/16)
[+ (xh/16)*Wls in 3-term mode]. Q is stored to SBUF as an fp8 hi+lo pair
(bias folded into hi via tensor_scalar_add; lo via scalar_tensor_tensor),
K as fp8 hi. Scores: one DR instr per (head, ktile): slots
(Kh, Qh) + (Kh dup via 0-stride, Qlo). exp runs on Activation into bf16
left-packed per-head strips, merged across ktiles sharing a PSUM tile
(6 exp instrs per head). PV is "flipped": stationary = ets [k, q-block]
(full 128x128 array use), moving = V bf16 [k, 64] -> z [q, h] in a single
PSUM bank per q-tile incl. per-head denominator columns (ones moving).
Normalize = one reciprocal + one broadcast multiply per q-tile; z^T via
bf16 PE transposes; out-proj bf16; bf16 output (host converts to f32).
"""

import sys

sys.path.insert(0, "/opt/trn_rl_repo")

from contextlib import ExitStack

import numpy as np
import ml_dtypes

import concourse.bass as bass
import concourse.tile as tile
from concourse import bacc, mybir
from concourse.bass_utils import run_bass_kernel_spmd

B, S, D, N, H = 4, 1024, 768, 12, 64
NHC = 6            # heads per core
NPAIR = NHC // 2   # head pairs (2 heads -> 128 partitions)
HD = NHC * H       # 384 packed head dim per core
P = 128
NDT = D // P       # 6 d-tiles
NST = S // P       # 8 k-tiles
QB = 512           # strip width (q)
FW = 32.0          # host weight fold (power of 2)
F32 = mybir.dt.float32
F8 = mybir.dt.float8e4
BF = mybir.dt.bfloat16
DR = mybir.MatmulPerfMode.DoubleRow
EXP_SCALE = 1.0 / (np.sqrt(float(H)) * FW * FW)

QK_3T = False      # 3-term QK projections (False: 2-term, faster/less exact)

# wall column layout (fp8 weights, priority-ordered for DMA chunking)
def _wall_layout():
    cols = {}
    off = 0
    def put(name, w):
        nonlocal off
        cols[name] = (off, w)
        off += w
    # chunk 0: pair0 Q then K (two priority sub-chunks)
    for t in ("qh", "qhd") + (("qls",) if QK_3T else ()):
        put(t + "0", P)
    for t in ("kh", "khd") + (("kls",) if QK_3T else ()):
        put(t + "0", P)
    c0 = off
    # chunk 1: V (all pairs)
    for t in ("vh", "vhd", "vls"):
        put(t, HD)
    c1 = off
    # chunks 2, 3: pairs 1, 2
    for g in (1, 2):
        for t in ("qh", "kh", "qhd", "khd") + (("qls", "kls") if QK_3T else ()):
            put(t + str(g), P)
    return cols, (0, c0, c1, (c1 + off) // 2, off)

WALL_COLS, WALL_CHUNKS = _wall_layout()
WALL_W = WALL_CHUNKS[-1]

# ets strip segment offsets (left-packed live columns per ktile)
def _segs(strip):
    offs, off = [], 0
    nkt = 4 if strip == 0 else NST
    for kt in range(nkt):
        if strip == 0:
            live = QB - kt * P
        else:
            live = min(S - kt * P, QB)
        offs.append((off, live))
        off += live
    return offs, off

SEG0, LEN0 = _segs(0)
SEG1, LEN1 = _segs(1)
# exp merge groups: lists of (ktile list, psum cols used)
EXPG0 = [[0, 1], [2, 3]]
EXPG1 = [[0, 1], [2, 3], [4, 5], [6, 7]]

_CACHE = {}


def _build():
    nc = bacc.Bacc()
    wall_d = nc.declare_dram_parameter("wall", [D, WALL_W], F8, isOutput=False)
    xall_d = nc.declare_dram_parameter("xall", [3, D, S], F8, isOutput=False)
    wo_d = nc.declare_dram_parameter("wo", [HD, D], BF, isOutput=False)
    bq_d = nc.declare_dram_parameter("bq", [P, NPAIR], F32, isOutput=False)
    trid_d = nc.declare_dram_parameter("trid", [P, 2 * P], BF, isOutput=False)
    out_d = nc.declare_dram_parameter("out", [S, D], BF, isOutput=True)
    import os
    DBG = os.environ.get("KDBG") == "1"
    if DBG:
        dq_d = nc.declare_dram_parameter("dq", [P, NPAIR * 2 * S], F8, isOutput=True)
        dk_d = nc.declare_dram_parameter("dk", [P, NPAIR * S], F8, isOutput=True)
        dets_d = nc.declare_dram_parameter("dets", [P, NHC * 3328], BF, isOutput=True)
        dva_d = nc.declare_dram_parameter("dva", [P, NST * NHC * H], BF, isOutput=True)

    wall_r = wall_d[:].rearrange("(t p) c -> p t c", p=P)
    xall_r = xall_d[:].rearrange("i (t p) s -> p i t s", p=P)
    wo_r = wo_d[:].rearrange("(t p) d -> p t d", p=P)

    with tile.TileContext(nc) as tc, ExitStack() as ctx:
        consts = ctx.enter_context(tc.tile_pool(name="consts", bufs=1))
        persist = ctx.enter_context(tc.tile_pool(name="persist", bufs=1))
        znp = ctx.enter_context(tc.tile_pool(name="znp", bufs=2))
        zntp = ctx.enter_context(tc.tile_pool(name="zntp", bufs=2))
        outp = ctx.enter_context(tc.tile_pool(name="outp", bufs=2))
        smalls = ctx.enter_context(tc.tile_pool(name="smalls", bufs=2))

        wall = consts.tile([P, NDT, WALL_W], F8)
        xall = consts.tile([P, 3, NDT, S], F8)
        xh = xall[:, 0]
        xls = xall[:, 1]
        xhd = xall[:, 2]
        wo = consts.tile([P, NPAIR, D], BF)
        bq = consts.tile([P, NPAIR], F32)
        trid = consts.tile([P, 2 * P], BF)
        tri = trid[:, 0:P]
        ident = trid[:, P : 2 * P]

        def wslice(name, g, t0, nt, c0, cw):
            """stationary slice [P, nt, cw] of wall tensor `name` (pair g)."""
            base = WALL_COLS[name + str(g) if name[0] in "qk" else name][0]
            return wall[:, t0 : t0 + nt, base + c0 : base + c0 + cw]

        # ---- DMA priority order (all on SP/sync queue) ----
        def dma_wall_chunk(i):
            c0, c1 = WALL_CHUNKS[i], WALL_CHUNKS[i + 1]
            nc.sync.dma_start(out=wall[:, :, c0:c1], in_=wall_r[:, :, c0:c1])

        def dma_x(i0, i1, s2):
            nc.sync.dma_start(
                out=xall[:, i0:i1, :, s2 * QB : (s2 + 1) * QB],
                in_=xall_r[:, i0:i1, :, s2 * QB : (s2 + 1) * QB],
            )

        dma_wall_chunk(0)
        nc.sync.dma_start(out=bq, in_=bq_d[:])
        dma_x(0, 1, 0)         # xh half0
        nc.sync.dma_start(out=trid, in_=trid_d[:])
        dma_x(1, 2, 0)         # xls half0
        dma_x(2, 3, 0)         # xhd half0
        dma_wall_chunk(1)      # V weights
        c2, c3 = WALL_CHUNKS[2], WALL_CHUNKS[4]
        nc.sync.dma_start(out=wall[:, :, c2:c3], in_=wall_r[:, :, c2:c3])
        dma_x(0, 3, 1)         # all x half1, one DMA
        nc.sync.dma_start(out=wo, in_=wo_r)

        # ---- persistent activations ----
        qT8 = persist.tile([P, NPAIR, 2, S], F8)   # dim2: hi/lo
        kT8 = persist.tile([P, NPAIR, S], F8)
        vA = persist.tile([P, NST, NHC, H], BF)
        ets0 = persist.tile([P, NHC, LEN0], BF)
        ets1 = persist.tile([P, NHC, LEN1], BF)
        ones = persist.tile([P, 1], BF)
        nc.gpsimd.memset(ones, 1.0)

        ps_sm = ctx.enter_context(tc.tile_pool(name="ps_sm", bufs=2, space="PSUM"))
        ps_z = ctx.enter_context(tc.tile_pool(name="ps_z", bufs=2, space="PSUM"))
        ps_m = ctx.enter_context(tc.tile_pool(name="ps_m", bufs=2, space="PSUM"))

        # PE warm-up: carries the p-state ramp during the DMA prologue.
        dums = consts.tile([P, 2, QB], F8)
        nc.gpsimd.memset(dums, 0.0)
        # preload the Exp activation table off the critical path
        scr = smalls.tile([P, 1], BF, name="scr")
        nc.scalar.activation(scr, ones, mybir.ActivationFunctionType.Exp)
        wps = ps_m.tile([P, QB], F32, name="warm", tag="m")
        for i in range(10):
            nc.tensor.matmul(
                wps, dums[:, :, 0:P], dums, start=(i == 0), stop=(i == 9),
                perf_mode=DR,
            )

        def proj_qk(g, s2):
            """Q and K projections for pair g, s-half s2 (fp8 DR)."""
            s0 = s2 * QB
            k_on_act = (g == 0 and s2 == 0)
            for wname, ap_hi_lo in (("q", True), ("k", False)):
                pp = ps_m.tile([P, QB], F32, tag="m")
                ktp = [(t, t + 2) for t in (0, 2, 4)]
                for t, _ in ktp:
                    nc.tensor.matmul(
                        pp, wslice(wname + "h", g, t, 2, 0, P),
                        xh[:, t : t + 2, s0 : s0 + QB],
                        start=(t == 0), stop=False, perf_mode=DR)
                for i, (t, _) in enumerate(ktp):
                    nc.tensor.matmul(
                        pp, wslice(wname + "hd", g, t, 2, 0, P),
                        xls[:, t : t + 2, s0 : s0 + QB],
                        start=False, stop=(not QK_3T and i == 2),
                        perf_mode=DR)
                if QK_3T:
                    for i, (t, _) in enumerate(ktp):
                        nc.tensor.matmul(
                            pp, wslice(wname + "ls", g, t, 2, 0, P),
                            xhd[:, t : t + 2, s0 : s0 + QB],
                            start=False, stop=(i == 2), perf_mode=DR)
                if ap_hi_lo:
                    nc.vector.tensor_scalar_add(
                        qT8[:, g, 0, s0 : s0 + QB], pp, bq[:, g : g + 1])
                    nc.vector.scalar_tensor_tensor(
                        out=qT8[:, g, 1, s0 : s0 + QB], in0=pp,
                        scalar=bq[:, g : g + 1], in1=qT8[:, g, 0, s0 : s0 + QB],
                        op0=mybir.AluOpType.add, op1=mybir.AluOpType.subtract)
                elif k_on_act:
                    nc.scalar.copy(kT8[:, g, s0 : s0 + QB], pp)
                else:
                    nc.vector.tensor_copy(out=kT8[:, g, s0 : s0 + QB], in_=pp)

        def proj_v(st):
            """V projection for s-tile st -> vA bf16 (fp8 DR, 3-term)."""
            pp = ps_m.tile([P, HD], F32, tag="m")
            for t in (0, 2, 4):
                nc.tensor.matmul(
                    pp, xh[:, t : t + 2, st * P : (st + 1) * P],
                    wslice("vh", 0, t, 2, 0, HD),
                    start=(t == 0), stop=False, perf_mode=DR)
            for i, t in enumerate((0, 2, 4)):
                nc.tensor.matmul(
                    pp, xls[:, t : t + 2, st * P : (st + 1) * P],
                    wslice("vhd", 0, t, 2, 0, HD),
                    start=False, stop=False, perf_mode=DR)
            for i, t in enumerate((0, 2, 4)):
                nc.tensor.matmul(
                    pp, xhd[:, t : t + 2, st * P : (st + 1) * P],
                    wslice("vls", 0, t, 2, 0, HD),
                    start=False, stop=(i == 2), perf_mode=DR)
            nc.vector.tensor_copy(
                out=vA[:, st, :, :], in_=pp.rearrange("p (n h) -> p n h", n=NHC))

        def scores_head(h, strip):
            """All score DRs + merged exps + masks for head h of strip."""
            g, e = divmod(h, 2)
            hp = e * H
            ets = ets0 if strip == 0 else ets1
            segs = SEG0 if strip == 0 else SEG1
            q0 = strip * QB
            groups = EXPG0 if strip == 0 else EXPG1
            for kts in groups:
                used = sum(segs[kt][1] for kt in kts)
                sm = ps_sm.tile([P, 2 * QB], F32, tag="sm")
                pcol = 0
                for kt in kts:
                    live = segs[kt][1]
                    o = (q0 + QB - live) - q0  # live q starts at q0 + o
                    kst = kT8[hp : hp + H, g, kt * P : (kt + 1) * P]
                    kst = kst.rearrange("p (o m) -> p o m", o=1).broadcast_to((H, 2, P))
                    nc.tensor.matmul(
                        sm[:, pcol : pcol + live], kst,
                        qT8[hp : hp + H, g, :, q0 + o : q0 + QB],
                        start=True, stop=True, perf_mode=DR)
                    pcol += live
                seg0 = segs[kts[0]][0]
                nc.scalar.activation(
                    ets[:, h, seg0 : seg0 + used], sm[:, 0:used],
                    mybir.ActivationFunctionType.Exp, scale=EXP_SCALE)
            # diagonal masks (ets *= tri on first 128 stored cols of diag kts)
            dkts = range(4) if strip == 0 else range(4, 8)
            for kt in dkts:
                so = segs[kt][0]
                nc.gpsimd.tensor_mul(
                    ets[:, h, so : so + P], ets[:, h, so : so + P], tri)

        def pv_qtile(strip, j):
            """z psum for q-tile j of strip: all heads' PV + denominators."""
            ets = ets0 if strip == 0 else ets1
            segs = SEG0 if strip == 0 else SEG1
            q0 = strip * QB
            zp = ps_z.tile([P, 390], F32, tag="z")
            gq = q0 + j * P  # global q block start
            for h in range(NHC):
                kts = [kt for kt in range(len(segs))
                       if kt * P <= gq + P - 1 and True]
                # live ktiles: those whose k-range start <= last q of block
                kts = [kt for kt in range(len(segs)) if kt * P < gq + P]
                for i, kt in enumerate(kts):
                    o = max(kt * P - q0, 0)
                    col = segs[kt][0] + (j * P - o)
                    st = ets[:, h, col : col + P]
                    nc.tensor.matmul(
                        zp[:, h * H : (h + 1) * H], st, vA[:, kt, h, :],
                        start=(i == 0), stop=(i == len(kts) - 1))
                for i, kt in enumerate(kts):
                    o = max(kt * P - q0, 0)
                    col = segs[kt][0] + (j * P - o)
                    st = ets[:, h, col : col + P]
                    nc.tensor.matmul(
                        zp[:, 384 + h : 385 + h], st, ones,
                        start=(i == 0), stop=(i == len(kts) - 1))
            return zp

        def phase3(strip, j, zp, last):
            """normalize -> transpose -> out-proj -> store for q-tile j."""
            row0 = strip * QB + j * P
            r = smalls.tile([P, NHC], F32)
            nc.vector.reciprocal(r, zp[:, 384:390])
            zn = znp.tile([P, NHC, H], BF)
            rb = r.rearrange("p (h o) -> p h o", o=1).broadcast_to((P, NHC, H))
            nc.vector.tensor_mul(
                zn, zp[:, 0:384].rearrange("p (n h) -> p n h", n=NHC), rb)
            znt_ps = ps_m.tile([P, NPAIR, P], BF, tag="m")
            for t in range(NPAIR):
                nc.tensor.matmul(
                    znt_ps[:, t, :], zn[:, 2 * t : 2 * t + 2, :].rearrange(
                        "p n h -> p (n h)"),
                    ident, is_transpose=True)
            znt = zntp.tile([P, NPAIR, P], BF)
            nc.vector.tensor_copy(out=znt, in_=znt_ps)
            osb = outp.tile([P, D], BF)
            for dh in range(2):
                op = ps_m.tile([P, D // 2], F32, tag="m")
                for t in range(NPAIR):
                    nc.tensor.matmul(
                        op, znt[:, t, :],
                        wo[:, t, dh * (D // 2) : (dh + 1) * (D // 2)],
                        start=(t == 0), stop=(t == NPAIR - 1))
                sl = osb[:, dh * (D // 2) : (dh + 1) * (D // 2)]
                if last and dh == 0:
                    nc.scalar.copy(sl, op)
                else:
                    nc.vector.tensor_copy(out=sl, in_=op)
                nc.sync.dma_start(
                    out=out_d[row0 : row0 + P, dh * (D // 2) : (dh + 1) * (D // 2)],
                    in_=sl)

        def phase3_tail(zpA, zpB):
            """Last two q-tiles: interleave DVE/Act chains to shorten the tail."""
            rows = (QB + 2 * P, QB + 3 * P)
            rA = smalls.tile([P, NHC], F32, name="rA")
            rB = smalls.tile([P, NHC], F32, name="rB")
            nc.vector.reciprocal(rA, zpA[:, 384:390])
            nc.vector.reciprocal(rB, zpB[:, 384:390])
            zns, znts = [], []
            for nm, zp, r in (("A", zpA, rA), ("B", zpB, rB)):
                zn = znp.tile([P, NHC, H], BF, name="znt_" + nm)
                rb = r.rearrange("p (h o) -> p h o", o=1).broadcast_to((P, NHC, H))
                nc.vector.tensor_mul(
                    zn, zp[:, 0:384].rearrange("p (n h) -> p n h", n=NHC), rb)
                zns.append(zn)
            for nm, zn in zip("AB", zns):
                znt_ps = ps_m.tile([P, NPAIR, P], BF, tag="m", name="znp_" + nm)
                for t in range(NPAIR):
                    nc.tensor.matmul(
                        znt_ps[:, t, :], zn[:, 2 * t : 2 * t + 2, :].rearrange(
                            "p n h -> p (n h)"),
                        ident, is_transpose=True)
                znt = zntp.tile([P, NPAIR, P], BF, name="zntt_" + nm)
                if nm == "A":
                    nc.scalar.copy(znt, znt_ps)
                else:
                    nc.vector.tensor_copy(out=znt, in_=znt_ps)
                znts.append(znt)
            for dh in range(2):
                for i, znt in enumerate(znts):
                    op = ps_m.tile([P, D // 2], F32, tag="m", name=f"op_{i}{dh}")
                    for t in range(NPAIR):
                        nc.tensor.matmul(
                            op, znt[:, t, :],
                            wo[:, t, dh * (D // 2) : (dh + 1) * (D // 2)],
                            start=(t == 0), stop=(t == NPAIR - 1))
                    osb = outp.tile([P, D // 2], BF, name=f"osb_{i}{dh}")
                    if (i + dh) % 2 == 0:
                        nc.scalar.copy(osb, op)
                    else:
                        nc.vector.tensor_copy(out=osb, in_=op)
                    nc.sync.dma_start(
                        out=out_d[rows[i] : rows[i] + P,
                                  dh * (D // 2) : (dh + 1) * (D // 2)],
                        in_=osb)

        # ================= schedule =================
        proj_qk(0, 0)
        scores_head(0, 0)
        scores_head(1, 0)
        proj_qk(1, 0)
        scores_head(2, 0)
        scores_head(3, 0)
        proj_qk(2, 0)
        scores_head(4, 0)
        scores_head(5, 0)
        proj_qk(0, 1)
        scores_head(0, 1)
        scores_head(1, 1)
        for st in range(4):
            proj_v(st)
        zp0 = pv_qtile(0, 0)
        proj_qk(1, 1)
        scores_head(2, 1)
        scores_head(3, 1)
        zp1 = pv_qtile(0, 1)
        phase3(0, 0, zp0, False)
        proj_qk(2, 1)
        scores_head(4, 1)
        scores_head(5, 1)
        zp2 = pv_qtile(0, 2)
        phase3(0, 1, zp1, False)
        for st in range(4, NST):
            proj_v(st)
        zp3 = pv_qtile(0, 3)
        phase3(0, 2, zp2, False)
        zp4 = pv_qtile(1, 0)
        phase3(0, 3, zp3, False)
        zp5 = pv_qtile(1, 1)
        phase3(1, 0, zp4, True)
        zp6 = pv_qtile(1, 2)
        phase3(1, 1, zp5, True)
        zp7 = pv_qtile(1, 3)
        phase3_tail(zp6, zp7)
        if DBG:
            nc.sync.dma_start(out=dq_d[:], in_=qT8.rearrange("p a b c -> p (a b c)"))
            nc.sync.dma_start(out=dk_d[:], in_=kT8.rearrange("p a b -> p (a b)"))
            nc.sync.dma_start(out=dets_d[:], in_=ets1.rearrange("p a b -> p (a b)"))
            nc.sync.dma_start(out=dva_d[:], in_=vA.rearrange("p a b c -> p (a b c)"))

    if not nc.is_finalized():
        nc.finalize()
    return nc


def _get_program():
    if "nc" not in _CACHE:
        _CACHE["nc"] = _build()
    return _CACHE["nc"]


F8NP = ml_dtypes.float8_e4m3
BFNP = ml_dtypes.bfloat16


def _q8(a):
    return a.astype(F8NP)


def make_in_maps(normalized_resid_pre, W_Q, W_K, W_V, b_Q):
    x = np.asarray(normalized_resid_pre, np.float32)
    W_Q = np.asarray(W_Q, np.float32) * FW
    W_K = np.asarray(W_K, np.float32) * FW
    W_V = np.asarray(W_V, np.float32) * FW
    b_Q = np.asarray(b_Q, np.float32) * FW

    tri = np.triu(np.ones((P, P), np.float32)).astype(BFNP)
    ident = np.eye(P, dtype=np.float32).astype(BFNP)
    trid = np.concatenate([tri, ident], axis=1)

    in_maps = []
    for c in range(8):
        b, hg = divmod(c, 2)
        hs = slice(hg * NHC, (hg + 1) * NHC)
        xt = np.ascontiguousarray(x[b].T)          # [D, S]
        xh = _q8(xt)
        xhf = xh.astype(np.float32)
        xls = _q8((xt - xhf) * 16.0)
        xhd = (xhf / 16.0).astype(F8NP)            # exact exponent shift
        xall = np.stack([xh, xls, xhd], axis=0)    # [3, D, S]

        wall = np.zeros((D, WALL_W), F8NP)
        for wname, W in (("q", W_Q), ("k", W_K), ("v", W_V)):
            Wg = W[hs].transpose(1, 0, 2).reshape(D, HD)  # [D, 6*64]
            Wh = _q8(Wg)
            Whf = Wh.astype(np.float32)
            Whd = (Whf / 16.0).astype(F8NP)
            Wls = _q8((Wg - Whf) * 16.0)
            if wname == "v":
                c0 = WALL_COLS["vh"][0]
                wall[:, c0 : c0 + HD] = Wh
                c0 = WALL_COLS["vhd"][0]
                wall[:, c0 : c0 + HD] = Whd
                c0 = WALL_COLS["vls"][0]
                wall[:, c0 : c0 + HD] = Wls
            else:
                for g in range(NPAIR):
                    sl = slice(g * P, (g + 1) * P)
                    c0 = WALL_COLS[wname + "h" + str(g)][0]
                    wall[:, c0 : c0 + P] = Wh[:, sl]
                    c0 = WALL_COLS[wname + "hd" + str(g)][0]
                    wall[:, c0 : c0 + P] = Whd[:, sl]
                    if QK_3T:
                        c0 = WALL_COLS[wname + "ls" + str(g)][0]
                        wall[:, c0 : c0 + P] = Wls[:, sl]

        in_maps.append({
            "wall": wall,
            "xall": xall,
            "wo": None,  # filled in kernel()
            "bq": np.ascontiguousarray(b_Q[hs].reshape(NPAIR, P).T),
            "trid": trid,
        })
    return in_maps


def kernel(
    normalized_resid_pre, W_Q, W_K, W_V, W_O, b_Q, b_K, b_V, b_O, **_unused
):
    W_O = np.asarray(W_O, np.float32)
    b_V, b_O = np.asarray(b_V, np.float32), np.asarray(b_O, np.float32)
    in_maps = make_in_maps(normalized_resid_pre, W_Q, W_K, W_V, b_Q)
    for c in range(8):
        hg = c % 2
        hs = slice(hg * NHC, (hg + 1) * NHC)
        in_maps[c]["wo"] = np.ascontiguousarray(
            (W_O[hs].reshape(HD, D) / FW).astype(BFNP))

    nc = _get_program()
    res = run_bass_kernel_spmd(nc, in_maps, list(range(8))).results

    out = np.zeros((B, S, D), np.float32)
    for c in range(8):
        out[c // 2] += res[c]["out"].astype(np.float32)
    out += b_O + np.einsum("nh,nhd->d", b_V, W_O)
    return out


# revision 21
# speedup vs baseline: 1.0539x; 1.0491x over previous
"""Causal multi-head attention block on 8 NeuronCores (Trainium2, Bass/Tile).

Reference computation (per batch b):
  Q = x @ W_Q + b_Q ; K = x @ W_K (+ b_K, dropped: softmax shift-invariant)
  scores = Q K^T / sqrt(H); causal mask; probs = softmax(scores)
  out = (probs @ V) @ W_O + b_O   (b_V folded on host: probs rows sum to 1)

Sharding: core c -> batch c//2, head-group c%2 (6 of 12 heads). Host sums
the two head-group partials per batch and adds b_O + sum_nh b_V[n,h]*W_O.

Numerics (fp8 DoubleRow): weights are host-folded x32 so fp8 residual
splits stay in e4m3's normal range. Projections run as fp8 DoubleRow
matmuls (0.5 cy/row, 2 k-tiles per instr): Q/K/V = xh*Wh + xls*(Wh/16)
[+ (xh/16)*Wls in 3-term mode]. Q is stored to SBUF as an fp8 hi+lo pair
(bias folded into hi via tensor_scalar_add; lo via scalar_tensor_tensor),
K as fp8 hi. Scores: one DR instr per (head, ktile): slots
(Kh, Qh) + (Kh dup via 0-stride, Qlo). exp runs on Activation into bf16
left-packed per-head strips, merged across ktiles sharing a PSUM tile
(6 exp instrs per head). PV is "flipped": stationary = ets [k, q-block]
(full 128x128 array use), moving = V bf16 [k, 64] -> z [q, h] in a single
PSUM bank per q-tile incl. per-head denominator columns (ones moving).
Normalize = one reciprocal + one broadcast multiply per q-tile; z^T via
bf16 PE transposes; out-proj bf16; bf16 output (host converts to f32).
"""

import sys

sys.path.insert(0, "/opt/trn_rl_repo")

from contextlib import ExitStack

import numpy as np
import ml_dtypes

import concourse.bass as bass
import concourse.tile as tile
from concourse import bacc, mybir
from concourse.bass_utils import run_bass_kernel_spmd

B, S, D, N, H = 4, 1024, 768, 12, 64
NHC = 6            # heads per core
NPAIR = NHC // 2   # head pairs (2 heads -> 128 partitions)
HD = NHC * H       # 384 packed head dim per core
P = 128
NDT = D // P       # 6 d-tiles
NST = S // P       # 8 k-tiles
QB = 512           # strip width (q)
FW = 32.0          # host weight fold (power of 2)
F32 = mybir.dt.float32
F8 = mybir.dt.float8e4
BF = mybir.dt.bfloat16
DR = mybir.MatmulPerfMode.DoubleRow
EXP_SCALE = 1.0 / (np.sqrt(float(H)) * FW * FW)

QK_3T = False      # 3-term QK projections (False: 2-term, faster/less exact)

# wall column layout (fp8 weights, priority-ordered for DMA chunking)
def _wall_layout():
    cols = {}
    off = 0
    def put(name, w):
        nonlocal off
        cols[name] = (off, w)
        off += w
    # chunk 0: pair0 Q then K (two priority sub-chunks)
    for t in ("qh", "qhd") + (("qls",) if QK_3T else ()):
        put(t + "0", P)
    for t in ("kh", "khd") + (("kls",) if QK_3T else ()):
        put(t + "0", P)
    c0 = off
    # chunk 1: V (all pairs)
    for t in ("vh", "vhd", "vls"):
        put(t, HD)
    c1 = off
    # chunks 2, 3: pairs 1, 2
    for g in (1, 2):
        for t in ("qh", "kh", "qhd", "khd") + (("qls", "kls") if QK_3T else ()):
            put(t + str(g), P)
    return cols, (0, c0, c1, (c1 + off) // 2, off)

WALL_COLS, WALL_CHUNKS = _wall_layout()
WALL_W = WALL_CHUNKS[-1]

# ets strip segment offsets (left-packed live columns per ktile)
def _segs(strip):
    offs, off = [], 0
    nkt = 4 if strip == 0 else NST
    for kt in range(nkt):
        if strip == 0:
            live = QB - kt * P
        else:
            live = min(S - kt * P, QB)
        offs.append((off, live))
        off += live
    return offs, off

SEG0, LEN0 = _segs(0)
SEG1, LEN1 = _segs(1)
# exp merge groups: lists of (ktile list, psum cols used)
EXPG0 = [[0, 1], [2, 3]]
EXPG1 = [[0, 1], [2, 3], [4, 5], [6, 7]]

_CACHE = {}


def _build():
    nc = bacc.Bacc()
    wall_d = nc.declare_dram_parameter("wall", [D, WALL_W], F8, isOutput=False)
    xh_d = nc.declare_dram_parameter("xh", [D, S], F8, isOutput=False)
    xls_d = nc.declare_dram_parameter("xls", [D, S], F8, isOutput=False)
    xhd_d = nc.declare_dram_parameter("xhd", [D, S], F8, isOutput=False)
    wo_d = nc.declare_dram_parameter("wo", [HD, D], BF, isOutput=False)
    bq_d = nc.declare_dram_parameter("bq", [P, NPAIR], F32, isOutput=False)
    trid_d = nc.declare_dram_parameter("trid", [P, 2 * P], BF, isOutput=False)
    out_d = nc.declare_dram_parameter("out", [S, D], BF, isOutput=True)
    import os
    DBG = os.environ.get("KDBG") == "1"
    if DBG:
        dq_d = nc.declare_dram_parameter("dq", [P, NPAIR * 2 * S], F8, isOutput=True)
        dk_d = nc.declare_dram_parameter("dk", [P, NPAIR * S], F8, isOutput=True)
        dets_d = nc.declare_dram_parameter("dets", [P, NHC * 3328], BF, isOutput=True)
        dva_d = nc.declare_dram_parameter("dva", [P, NST * NHC * H], BF, isOutput=True)

    wall_r = wall_d[:].rearrange("(t p) c -> p t c", p=P)
    xh_r = xh_d[:].rearrange("(t p) s -> p t s", p=P)
    xls_r = xls_d[:].rearrange("(t p) s -> p t s", p=P)
    xhd_r = xhd_d[:].rearrange("(t p) s -> p t s", p=P)
    wo_r = wo_d[:].rearrange("(t p) d -> p t d", p=P)

    with tile.TileContext(nc) as tc, ExitStack() as ctx:
        consts = ctx.enter_context(tc.tile_pool(name="consts", bufs=1))
        persist = ctx.enter_context(tc.tile_pool(name="persist", bufs=1))
        znp = ctx.enter_context(tc.tile_pool(name="znp", bufs=2))
        zntp = ctx.enter_context(tc.tile_pool(name="zntp", bufs=2))
        outp = ctx.enter_context(tc.tile_pool(name="outp", bufs=2))
        smalls = ctx.enter_context(tc.tile_pool(name="smalls", bufs=2))

        wall = consts.tile([P, NDT, WALL_W], F8)
        xh = consts.tile([P, NDT, S], F8)
        xls = consts.tile([P, NDT, S], F8)
        xhd = consts.tile([P, NDT, S], F8, name="xhd")
        wo = consts.tile([P, NPAIR, D], BF)
        bq = consts.tile([P, NPAIR], F32)
        trid = consts.tile([P, 2 * P], BF)
        tri = trid[:, 0:P]
        ident = trid[:, P : 2 * P]

        def wslice(name, g, t0, nt, c0, cw):
            """stationary slice [P, nt, cw] of wall tensor `name` (pair g)."""
            base = WALL_COLS[name + str(g) if name[0] in "qk" else name][0]
            return wall[:, t0 : t0 + nt, base + c0 : base + c0 + cw]

        # ---- DMA priority order (all on SP/sync queue) ----
        def dma_wall_chunk(i):
            c0, c1 = WALL_CHUNKS[i], WALL_CHUNKS[i + 1]
            nc.sync.dma_start(out=wall[:, :, c0:c1], in_=wall_r[:, :, c0:c1])

        def dma_x_half(sb_t, dram_r, s2):
            nc.sync.dma_start(
                out=sb_t[:, :, s2 * QB : (s2 + 1) * QB],
                in_=dram_r[:, :, s2 * QB : (s2 + 1) * QB],
            )

        dma_wall_chunk(0)
        nc.sync.dma_start(out=bq, in_=bq_d[:])
        dma_x_half(xh, xh_r, 0)
        nc.sync.dma_start(out=trid, in_=trid_d[:])
        dma_x_half(xls, xls_r, 0)
        dma_x_half(xhd, xhd_r, 0)
        dma_wall_chunk(1)      # V weights
        dma_wall_chunk(2)      # pair 1
        dma_wall_chunk(3)      # pair 2
        dma_x_half(xh, xh_r, 1)
        dma_x_half(xls, xls_r, 1)
        dma_x_half(xhd, xhd_r, 1)
        nc.sync.dma_start(out=wo, in_=wo_r)

        # ---- persistent activations ----
        qT8 = persist.tile([P, NPAIR, 2, S], F8)   # dim2: hi/lo
        kT8 = persist.tile([P, NPAIR, S], F8)
        vA = persist.tile([P, NST, NHC, H], BF)
        ets0 = persist.tile([P, NHC, LEN0], BF)
        ets1 = persist.tile([P, NHC, LEN1], BF)
        ones = persist.tile([P, 1], BF)
        nc.gpsimd.memset(ones, 1.0)

        ps_sm = ctx.enter_context(tc.tile_pool(name="ps_sm", bufs=2, space="PSUM"))
        ps_z = ctx.enter_context(tc.tile_pool(name="ps_z", bufs=2, space="PSUM"))
        ps_m = ctx.enter_context(tc.tile_pool(name="ps_m", bufs=2, space="PSUM"))

        # PE warm-up: carries the p-state ramp during the DMA prologue.
        dums = consts.tile([P, 2, QB], F8)
        nc.gpsimd.memset(dums, 0.0)
        # preload the Exp activation table off the critical path
        scr = smalls.tile([P, 1], BF, name="scr")
        nc.scalar.activation(scr, ones, mybir.ActivationFunctionType.Exp)
        wps = ps_m.tile([P, QB], F32, name="warm", tag="m")
        for i in range(10):
            nc.tensor.matmul(
                wps, dums[:, :, 0:P], dums, start=(i == 0), stop=(i == 9),
                perf_mode=DR,
            )

        def proj_qk(g, s2):
            """Q and K projections for pair g, s-half s2 (fp8 DR)."""
            s0 = s2 * QB
            k_on_act = (g == 0 and s2 == 0)
            for wname, ap_hi_lo in (("q", True), ("k", False)):
                pp = ps_m.tile([P, QB], F32, tag="m")
                ktp = [(t, t + 2) for t in (0, 2, 4)]
                for t, _ in ktp:
                    nc.tensor.matmul(
                        pp, wslice(wname + "h", g, t, 2, 0, P),
                        xh[:, t : t + 2, s0 : s0 + QB],
                        start=(t == 0), stop=False, perf_mode=DR)
                for i, (t, _) in enumerate(ktp):
                    nc.tensor.matmul(
                        pp, wslice(wname + "hd", g, t, 2, 0, P),
                        xls[:, t : t + 2, s0 : s0 + QB],
                        start=False, stop=(not QK_3T and i == 2),
                        perf_mode=DR)
                if QK_3T:
                    for i, (t, _) in enumerate(ktp):
                        nc.tensor.matmul(
                            pp, wslice(wname + "ls", g, t, 2, 0, P),
                            xhd[:, t : t + 2, s0 : s0 + QB],
                            start=False, stop=(i == 2), perf_mode=DR)
                if ap_hi_lo:
                    nc.vector.tensor_scalar_add(
                        qT8[:, g, 0, s0 : s0 + QB], pp, bq[:, g : g + 1])
                    nc.vector.scalar_tensor_tensor(
                        out=qT8[:, g, 1, s0 : s0 + QB], in0=pp,
                        scalar=bq[:, g : g + 1], in1=qT8[:, g, 0, s0 : s0 + QB],
                        op0=mybir.AluOpType.add, op1=mybir.AluOpType.subtract)
                elif k_on_act:
                    nc.scalar.copy(kT8[:, g, s0 : s0 + QB], pp)
                else:
                    nc.vector.tensor_copy(out=kT8[:, g, s0 : s0 + QB], in_=pp)

        def proj_v(st):
            """V projection for s-tile st -> vA bf16 (fp8 DR, 3-term)."""
            pp = ps_m.tile([P, HD], F32, tag="m")
            for t in (0, 2, 4):
                nc.tensor.matmul(
                    pp, xh[:, t : t + 2, st * P : (st + 1) * P],
                    wslice("vh", 0, t, 2, 0, HD),
                    start=(t == 0), stop=False, perf_mode=DR)
            for i, t in enumerate((0, 2, 4)):
                nc.tensor.matmul(
                    pp, xls[:, t : t + 2, st * P : (st + 1) * P],
                    wslice("vhd", 0, t, 2, 0, HD),
                    start=False, stop=False, perf_mode=DR)
            for i, t in enumerate((0, 2, 4)):
                nc.tensor.matmul(
                    pp, xhd[:, t : t + 2, st * P : (st + 1) * P],
                    wslice("vls", 0, t, 2, 0, HD),
                    start=False, stop=(i == 2), perf_mode=DR)
            nc.vector.tensor_copy(
                out=vA[:, st, :, :], in_=pp.rearrange("p (n h) -> p n h", n=NHC))

        def scores_head(h, strip):
            """All score DRs + merged exps + masks for head h of strip."""
            g, e = divmod(h, 2)
            hp = e * H
            ets = ets0 if strip == 0 else ets1
            segs = SEG0 if strip == 0 else SEG1
            q0 = strip * QB
            groups = EXPG0 if strip == 0 else EXPG1
            for kts in groups:
                used = sum(segs[kt][1] for kt in kts)
                sm = ps_sm.tile([P, 2 * QB], F32, tag="sm")
                pcol = 0
                for kt in kts:
                    live = segs[kt][1]
                    o = (q0 + QB - live) - q0  # live q starts at q0 + o
                    kst = kT8[hp : hp + H, g, kt * P : (kt + 1) * P]
                    kst = kst.rearrange("p (o m) -> p o m", o=1).broadcast_to((H, 2, P))
                    nc.tensor.matmul(
                        sm[:, pcol : pcol + live], kst,
                        qT8[hp : hp + H, g, :, q0 + o : q0 + QB],
                        start=True, stop=True, perf_mode=DR)
                    pcol += live
                seg0 = segs[kts[0]][0]
                nc.scalar.activation(
                    ets[:, h, seg0 : seg0 + used], sm[:, 0:used],
                    mybir.ActivationFunctionType.Exp, scale=EXP_SCALE)
            # diagonal masks (ets *= tri on first 128 stored cols of diag kts)
            dkts = range(4) if strip == 0 else range(4, 8)
            for kt in dkts:
                so = segs[kt][0]
                nc.gpsimd.tensor_mul(
                    ets[:, h, so : so + P], ets[:, h, so : so + P], tri)

        def pv_qtile(strip, j):
            """z psum for q-tile j of strip: all heads' PV + denominators."""
            ets = ets0 if strip == 0 else ets1
            segs = SEG0 if strip == 0 else SEG1
            q0 = strip * QB
            zp = ps_z.tile([P, 390], F32, tag="z")
            gq = q0 + j * P  # global q block start
            for h in range(NHC):
                kts = [kt for kt in range(len(segs))
                       if kt * P <= gq + P - 1 and True]
                # live ktiles: those whose k-range start <= last q of block
                kts = [kt for kt in range(len(segs)) if kt * P < gq + P]
                for i, kt in enumerate(kts):
                    o = max(kt * P - q0, 0)
                    col = segs[kt][0] + (j * P - o)
                    st = ets[:, h, col : col + P]
                    nc.tensor.matmul(
                        zp[:, h * H : (h + 1) * H], st, vA[:, kt, h, :],
                        start=(i == 0), stop=(i == len(kts) - 1))
                for i, kt in enumerate(kts):
                    o = max(kt * P - q0, 0)
                    col = segs[kt][0] + (j * P - o)
                    st = ets[:, h, col : col + P]
                    nc.tensor.matmul(
                        zp[:, 384 + h : 385 + h], st, ones,
                        start=(i == 0), stop=(i == len(kts) - 1))
            return zp

        def phase3(strip, j, zp, last):
            """normalize -> transpose -> out-proj -> store for q-tile j."""
            row0 = strip * QB + j * P
            r = smalls.tile([P, NHC], F32)
            nc.vector.reciprocal(r, zp[:, 384:390])
            zn = znp.tile([P, NHC, H], BF)
            rb = r.rearrange("p (h o) -> p h o", o=1).broadcast_to((P, NHC, H))
            nc.vector.tensor_mul(
                zn, zp[:, 0:384].rearrange("p (n h) -> p n h", n=NHC), rb)
            znt_ps = ps_m.tile([P, NPAIR, P], BF, tag="m")
            for t in range(NPAIR):
                nc.tensor.matmul(
                    znt_ps[:, t, :], zn[:, 2 * t : 2 * t + 2, :].rearrange(
                        "p n h -> p (n h)"),
                    ident, is_transpose=True)
            znt = zntp.tile([P, NPAIR, P], BF)
            nc.vector.tensor_copy(out=znt, in_=znt_ps)
            osb = outp.tile([P, D], BF)
            for dh in range(2):
                op = ps_m.tile([P, D // 2], F32, tag="m")
                for t in range(NPAIR):
                    nc.tensor.matmul(
                        op, znt[:, t, :],
                        wo[:, t, dh * (D // 2) : (dh + 1) * (D // 2)],
                        start=(t == 0), stop=(t == NPAIR - 1))
                sl = osb[:, dh * (D // 2) : (dh + 1) * (D // 2)]
                if last and dh == 0:
                    nc.scalar.copy(sl, op)
                else:
                    nc.vector.tensor_copy(out=sl, in_=op)
                nc.sync.dma_start(
                    out=out_d[row0 : row0 + P, dh * (D // 2) : (dh + 1) * (D // 2)],
                    in_=sl)

        def phase3_tail(zpA, zpB):
            """Last two q-tiles: interleave DVE/Act chains to shorten the tail."""
            rows = (QB + 2 * P, QB + 3 * P)
            rA = smalls.tile([P, NHC], F32, name="rA")
            rB = smalls.tile([P, NHC], F32, name="rB")
            nc.vector.reciprocal(rA, zpA[:, 384:390])
            nc.vector.reciprocal(rB, zpB[:, 384:390])
            zns, znts = [], []
            for nm, zp, r in (("A", zpA, rA), ("B", zpB, rB)):
                zn = znp.tile([P, NHC, H], BF, name="znt_" + nm)
                rb = r.rearrange("p (h o) -> p h o", o=1).broadcast_to((P, NHC, H))
                nc.vector.tensor_mul(
                    zn, zp[:, 0:384].rearrange("p (n h) -> p n h", n=NHC), rb)
                zns.append(zn)
            for nm, zn in zip("AB", zns):
                znt_ps = ps_m.tile([P, NPAIR, P], BF, tag="m", name="znp_" + nm)
                for t in range(NPAIR):
                    nc.tensor.matmul(
                        znt_ps[:, t, :], zn[:, 2 * t : 2 * t + 2, :].rearrange(
                            "p n h -> p (n h)"),
                        ident, is_transpose=True)
                znt = zntp.tile([P, NPAIR, P], BF, name="zntt_" + nm)
                if nm == "A":
                    nc.scalar.copy(znt, znt_ps)
                else:
                    nc.vector.tensor_copy(out=znt, in_=znt_ps)
                znts.append(znt)
            for dh in range(2):
                for i, znt in enumerate(znts):
                    op = ps_m.tile([P, D // 2], F32, tag="m", name=f"op_{i}{dh}")
                    for t in range(NPAIR):
                        nc.tensor.matmul(
                            op, znt[:, t, :],
                            wo[:, t, dh * (D // 2) : (dh + 1) * (D // 2)],
                            start=(t == 0), stop=(t == NPAIR - 1))
                    osb = outp.tile([P, D // 2], BF, name=f"osb_{i}{dh}")
                    if (i + dh) % 2 == 0:
                        nc.scalar.copy(osb, op)
                    else:
                        nc.vector.tensor_copy(out=osb, in_=op)
                    nc.sync.dma_start(
                        out=out_d[rows[i] : rows[i] + P,
                                  dh * (D // 2) : (dh + 1) * (D // 2)],
                        in_=osb)

        # ================= schedule =================
        proj_qk(0, 0)
        scores_head(0, 0)
        scores_head(1, 0)
        proj_qk(1, 0)
        scores_head(2, 0)
        scores_head(3, 0)
        proj_qk(2, 0)
        scores_head(4, 0)
        scores_head(5, 0)
        proj_qk(0, 1)
        scores_head(0, 1)
        scores_head(1, 1)
        for st in range(4):
            proj_v(st)
        zp0 = pv_qtile(0, 0)
        proj_qk(1, 1)
        scores_head(2, 1)
        scores_head(3, 1)
        zp1 = pv_qtile(0, 1)
        phase3(0, 0, zp0, False)
        proj_qk(2, 1)
        scores_head(4, 1)
        scores_head(5, 1)
        zp2 = pv_qtile(0, 2)
        phase3(0, 1, zp1, False)
        for st in range(4, NST):
            proj_v(st)
        zp3 = pv_qtile(0, 3)
        phase3(0, 2, zp2, False)
        zp4 = pv_qtile(1, 0)
        phase3(0, 3, zp3, False)
        zp5 = pv_qtile(1, 1)
        phase3(1, 0, zp4, True)
        zp6 = pv_qtile(1, 2)
        phase3(1, 1, zp5, True)
        zp7 = pv_qtile(1, 3)
        phase3_tail(zp6, zp7)
        if DBG:
            nc.sync.dma_start(out=dq_d[:], in_=qT8.rearrange("p a b c -> p (a b c)"))
            nc.sync.dma_start(out=dk_d[:], in_=kT8.rearrange("p a b -> p (a b)"))
            nc.sync.dma_start(out=dets_d[:], in_=ets1.rearrange("p a b -> p (a b)"))
            nc.sync.dma_start(out=dva_d[:], in_=vA.rearrange("p a b c -> p (a b c)"))

    if not nc.is_finalized():
        nc.finalize()
    return nc


def _get_program():
    if "nc" not in _CACHE:
        _CACHE["nc"] = _build()
    return _CACHE["nc"]


F8NP = ml_dtypes.float8_e4m3
BFNP = ml_dtypes.bfloat16


def _q8(a):
    return a.astype(F8NP)


def make_in_maps(normalized_resid_pre, W_Q, W_K, W_V, b_Q):
    x = np.asarray(normalized_resid_pre, np.float32)
    W_Q = np.asarray(W_Q, np.float32) * FW
    W_K = np.asarray(W_K, np.float32) * FW
    W_V = np.asarray(W_V, np.float32) * FW
    b_Q = np.asarray(b_Q, np.float32) * FW

    tri = np.triu(np.ones((P, P), np.float32)).astype(BFNP)
    ident = np.eye(P, dtype=np.float32).astype(BFNP)
    trid = np.concatenate([tri, ident], axis=1)

    in_maps = []
    for c in range(8):
        b, hg = divmod(c, 2)
        hs = slice(hg * NHC, (hg + 1) * NHC)
        xt = np.ascontiguousarray(x[b].T)          # [D, S]
        xh = _q8(xt)
        xhf = xh.astype(np.float32)
        xls = _q8((xt - xhf) * 16.0)
        xhd = (xhf / 16.0).astype(F8NP)            # exact exponent shift

        wall = np.zeros((D, WALL_W), F8NP)
        for wname, W in (("q", W_Q), ("k", W_K), ("v", W_V)):
            Wg = W[hs].transpose(1, 0, 2).reshape(D, HD)  # [D, 6*64]
            Wh = _q8(Wg)
            Whf = Wh.astype(np.float32)
            Whd = (Whf / 16.0).astype(F8NP)
            Wls = _q8((Wg - Whf) * 16.0)
            if wname == "v":
                c0 = WALL_COLS["vh"][0]
                wall[:, c0 : c0 + HD] = Wh
                c0 = WALL_COLS["vhd"][0]
                wall[:, c0 : c0 + HD] = Whd
                c0 = WALL_COLS["vls"][0]
                wall[:, c0 : c0 + HD] = Wls
            else:
                for g in range(NPAIR):
                    sl = slice(g * P, (g + 1) * P)
                    c0 = WALL_COLS[wname + "h" + str(g)][0]
                    wall[:, c0 : c0 + P] = Wh[:, sl]
                    c0 = WALL_COLS[wname + "hd" + str(g)][0]
                    wall[:, c0 : c0 + P] = Whd[:, sl]
                    if QK_3T:
                        c0 = WALL_COLS[wname + "ls" + str(g)][0]
                        wall[:, c0 : c0 + P] = Wls[:, sl]

        in_maps.append({
            "wall": wall,
            "xh": xh,
            "xls": xls,
            "xhd": xhd,
            "wo": None,  # filled in kernel()
            "bq": np.ascontiguousarray(b_Q[hs].reshape(NPAIR, P).T),
            "trid": trid,
        })
    return in_maps


def kernel(
    normalized_resid_pre, W_Q, W_K, W_V, W_O, b_Q, b_K, b_V, b_O, **_unused
):
    W_O = np.asarray(W_O, np.float32)
    b_V, b_O = np.asarray(b_V, np.float32), np.asarray(b_O, np.float32)
    in_maps = make_in_maps(normalized_resid_pre, W_Q, W_K, W_V, b_Q)
    for c in range(8):
        hg = c % 2
        hs = slice(hg * NHC, (hg + 1) * NHC)
        in_maps[c]["wo"] = np.ascontiguousarray(
            (W_O[hs].reshape(HD, D) / FW).astype(BFNP))

    nc = _get_program()
    res = run_bass_kernel_spmd(nc, in_maps, list(range(8))).results

    out = np.zeros((B, S, D), np.float32)
    for c in range(8):
        out[c // 2] += res[c]["out"].astype(np.float32)
    out += b_O + np.einsum("nh,nhd->d", b_V, W_O)
    return out


# revision 22
# speedup vs baseline: 1.0598x; 1.0056x over previous
"""Causal multi-head attention block on 8 NeuronCores (Trainium2, Bass/Tile).

Reference computation (per batch b):
  Q = x @ W_Q + b_Q ; K = x @ W_K (+ b_K, dropped: softmax shift-invariant)
  scores = Q K^T / sqrt(H); causal mask; probs = softmax(scores)
  out = (probs @ V) @ W_O + b_O   (b_V folded on host: probs rows sum to 1)

Sharding: core c -> batch c//2, head-group c%2 (6 of 12 heads). Host sums
the two head-group partials per batch and adds b_O + sum_nh b_V[n,h]*W_O.

Numerics (fp8 DoubleRow): weights are host-folded x32 so fp8 residual
splits stay in e4m3's normal range. Projections run as fp8 DoubleRow
matmuls (0.5 cy/row, 2 k-tiles per instr): Q/K/V = xh*Wh + xls*(Wh/16)
[+ (xh/16)*Wls in 3-term mode]. Q is stored to SBUF as an fp8 hi+lo pair
(bias folded into hi via tensor_scalar_add; lo via scalar_tensor_tensor),
K as fp8 hi. Scores: one DR instr per (head, ktile): slots
(Kh, Qh) + (Kh dup via 0-stride, Qlo). exp runs on Activation into bf16
left-packed per-head strips, merged across ktiles sharing a PSUM tile
(6 exp instrs per head). PV is "flipped": stationary = ets [k, q-block]
(full 128x128 array use), moving = V bf16 [k, 64] -> z [q, h] in a single
PSUM bank per q-tile incl. per-head denominator columns (ones moving).
Normalize = one reciprocal + one broadcast multiply per q-tile; z^T via
bf16 PE transposes; out-proj bf16; bf16 output (host converts to f32).
"""

import sys

sys.path.insert(0, "/opt/trn_rl_repo")

from contextlib import ExitStack

import numpy as np
import ml_dtypes

import concourse.bass as bass
import concourse.tile as tile
from concourse import bacc, mybir
from concourse.bass_utils import run_bass_kernel_spmd

B, S, D, N, H = 4, 1024, 768, 12, 64
NHC = 6            # heads per core
NPAIR = NHC // 2   # head pairs (2 heads -> 128 partitions)
HD = NHC * H       # 384 packed head dim per core
P = 128
NDT = D // P       # 6 d-tiles
NST = S // P       # 8 k-tiles
QB = 512           # strip width (q)
FW = 32.0          # host weight fold (power of 2)
F32 = mybir.dt.float32
F8 = mybir.dt.float8e4
BF = mybir.dt.bfloat16
DR = mybir.MatmulPerfMode.DoubleRow
EXP_SCALE = 1.0 / (np.sqrt(float(H)) * FW * FW)

QK_3T = False      # 3-term QK projections (False: 2-term, faster/less exact)

# wall column layout (fp8 weights, priority-ordered for DMA chunking)
def _wall_layout():
    cols = {}
    off = 0
    def put(name, w):
        nonlocal off
        cols[name] = (off, w)
        off += w
    # chunk 0: pair0 Q then K (two priority sub-chunks)
    for t in ("qh", "qhd") + (("qls",) if QK_3T else ()):
        put(t + "0", P)
    for t in ("kh", "khd") + (("kls",) if QK_3T else ()):
        put(t + "0", P)
    c0 = off
    # chunk 1: V (all pairs)
    for t in ("vh", "vhd", "vls"):
        put(t, HD)
    c1 = off
    # chunks 2, 3: pairs 1, 2
    for g in (1, 2):
        for t in ("qh", "kh", "qhd", "khd") + (("qls", "kls") if QK_3T else ()):
            put(t + str(g), P)
    return cols, (0, c0, c1, (c1 + off) // 2, off)

WALL_COLS, WALL_CHUNKS = _wall_layout()
WALL_W = WALL_CHUNKS[-1]

# ets strip segment offsets (left-packed live columns per ktile)
def _segs(strip):
    offs, off = [], 0
    nkt = 4 if strip == 0 else NST
    for kt in range(nkt):
        if strip == 0:
            live = QB - kt * P
        else:
            live = min(S - kt * P, QB)
        offs.append((off, live))
        off += live
    return offs, off

SEG0, LEN0 = _segs(0)
SEG1, LEN1 = _segs(1)
# exp merge groups: lists of (ktile list, psum cols used)
EXPG0 = [[0, 1], [2, 3]]
EXPG1 = [[0, 1], [2, 3], [4, 5], [6, 7]]

_CACHE = {}


def _build():
    nc = bacc.Bacc()
    wall_d = nc.declare_dram_parameter("wall", [D, WALL_W], F8, isOutput=False)
    xh_d = nc.declare_dram_parameter("xh", [D, S], F8, isOutput=False)
    xls_d = nc.declare_dram_parameter("xls", [D, S], F8, isOutput=False)
    xhd_d = nc.declare_dram_parameter("xhd", [D, S], F8, isOutput=False)
    wo_d = nc.declare_dram_parameter("wo", [HD, D], BF, isOutput=False)
    bq_d = nc.declare_dram_parameter("bq", [P, NPAIR], F32, isOutput=False)
    trid_d = nc.declare_dram_parameter("trid", [P, 2 * P], BF, isOutput=False)
    out_d = nc.declare_dram_parameter("out", [S, D], BF, isOutput=True)
    import os
    DBG = os.environ.get("KDBG") == "1"
    if DBG:
        dq_d = nc.declare_dram_parameter("dq", [P, NPAIR * 2 * S], F8, isOutput=True)
        dk_d = nc.declare_dram_parameter("dk", [P, NPAIR * S], F8, isOutput=True)
        dets_d = nc.declare_dram_parameter("dets", [P, NHC * 3328], BF, isOutput=True)
        dva_d = nc.declare_dram_parameter("dva", [P, NST * NHC * H], BF, isOutput=True)

    wall_r = wall_d[:].rearrange("(t p) c -> p t c", p=P)
    xh_r = xh_d[:].rearrange("(t p) s -> p t s", p=P)
    xls_r = xls_d[:].rearrange("(t p) s -> p t s", p=P)
    xhd_r = xhd_d[:].rearrange("(t p) s -> p t s", p=P)
    wo_r = wo_d[:].rearrange("(t p) d -> p t d", p=P)

    with tile.TileContext(nc) as tc, ExitStack() as ctx:
        consts = ctx.enter_context(tc.tile_pool(name="consts", bufs=1))
        persist = ctx.enter_context(tc.tile_pool(name="persist", bufs=1))
        znp = ctx.enter_context(tc.tile_pool(name="znp", bufs=2))
        zntp = ctx.enter_context(tc.tile_pool(name="zntp", bufs=2))
        outp = ctx.enter_context(tc.tile_pool(name="outp", bufs=2))
        smalls = ctx.enter_context(tc.tile_pool(name="smalls", bufs=2))

        wall = consts.tile([P, NDT, WALL_W], F8)
        xh = consts.tile([P, NDT, S], F8)
        xls = consts.tile([P, NDT, S], F8)
        xhd = consts.tile([P, NDT, S], F8, name="xhd")
        wo = consts.tile([P, NPAIR, D], BF)
        bq = consts.tile([P, NPAIR], F32)
        trid = consts.tile([P, 2 * P], BF)
        tri = trid[:, 0:P]
        ident = trid[:, P : 2 * P]

        def wslice(name, g, t0, nt, c0, cw):
            """stationary slice [P, nt, cw] of wall tensor `name` (pair g)."""
            base = WALL_COLS[name + str(g) if name[0] in "qk" else name][0]
            return wall[:, t0 : t0 + nt, base + c0 : base + c0 + cw]

        # ---- DMA priority order (all on SP/sync queue) ----
        def dma_wall_chunk(i):
            c0, c1 = WALL_CHUNKS[i], WALL_CHUNKS[i + 1]
            nc.sync.dma_start(out=wall[:, :, c0:c1], in_=wall_r[:, :, c0:c1])

        def dma_x_half(sb_t, dram_r, s2):
            nc.sync.dma_start(
                out=sb_t[:, :, s2 * QB : (s2 + 1) * QB],
                in_=dram_r[:, :, s2 * QB : (s2 + 1) * QB],
            )

        dma_wall_chunk(0)
        nc.sync.dma_start(out=bq, in_=bq_d[:])
        dma_x_half(xh, xh_r, 0)
        nc.sync.dma_start(out=trid, in_=trid_d[:])
        dma_x_half(xls, xls_r, 0)
        dma_x_half(xhd, xhd_r, 0)
        dma_wall_chunk(1)      # V weights
        dma_wall_chunk(2)      # pair 1
        dma_wall_chunk(3)      # pair 2
        dma_x_half(xh, xh_r, 1)
        dma_x_half(xls, xls_r, 1)
        dma_x_half(xhd, xhd_r, 1)
        nc.sync.dma_start(out=wo, in_=wo_r)

        # ---- persistent activations ----
        qT8 = persist.tile([P, NPAIR, 2, S], F8)   # dim2: hi/lo
        kT8 = persist.tile([P, NPAIR, S], F8)
        vA = persist.tile([P, NST, NHC, H], BF)
        ets0 = persist.tile([P, NHC, LEN0], BF)
        ets1 = persist.tile([P, NHC, LEN1], BF)
        ones = persist.tile([P, 1], BF)
        nc.gpsimd.memset(ones, 1.0)

        ps_sm = ctx.enter_context(tc.tile_pool(name="ps_sm", bufs=2, space="PSUM"))
        ps_z = ctx.enter_context(tc.tile_pool(name="ps_z", bufs=2, space="PSUM"))
        ps_m = ctx.enter_context(tc.tile_pool(name="ps_m", bufs=2, space="PSUM"))

        # PE warm-up: carries the p-state ramp during the DMA prologue.
        dums = consts.tile([P, 2, QB], F8)
        nc.gpsimd.memset(dums, 0.0)
        # preload the Exp activation table off the critical path
        scr = smalls.tile([P, 1], BF, name="scr")
        nc.scalar.activation(scr, ones, mybir.ActivationFunctionType.Exp)
        wps = ps_m.tile([P, QB], F32, name="warm", tag="m")
        for i in range(32):
            nc.tensor.matmul(
                wps, dums[:, :, 0:P], dums, start=(i == 0), stop=(i == 31),
                perf_mode=DR,
            )

        def proj_qk(g, s2):
            """Q and K projections for pair g, s-half s2 (fp8 DR)."""
            s0 = s2 * QB
            k_on_act = (g == 0 and s2 == 0)
            for wname, ap_hi_lo in (("q", True), ("k", False)):
                pp = ps_m.tile([P, QB], F32, tag="m")
                ktp = [(t, t + 2) for t in (0, 2, 4)]
                for t, _ in ktp:
                    nc.tensor.matmul(
                        pp, wslice(wname + "h", g, t, 2, 0, P),
                        xh[:, t : t + 2, s0 : s0 + QB],
                        start=(t == 0), stop=False, perf_mode=DR)
                for i, (t, _) in enumerate(ktp):
                    nc.tensor.matmul(
                        pp, wslice(wname + "hd", g, t, 2, 0, P),
                        xls[:, t : t + 2, s0 : s0 + QB],
                        start=False, stop=(not QK_3T and i == 2),
                        perf_mode=DR)
                if QK_3T:
                    for i, (t, _) in enumerate(ktp):
                        nc.tensor.matmul(
                            pp, wslice(wname + "ls", g, t, 2, 0, P),
                            xhd[:, t : t + 2, s0 : s0 + QB],
                            start=False, stop=(i == 2), perf_mode=DR)
                if ap_hi_lo:
                    nc.vector.tensor_scalar_add(
                        qT8[:, g, 0, s0 : s0 + QB], pp, bq[:, g : g + 1])
                    nc.vector.scalar_tensor_tensor(
                        out=qT8[:, g, 1, s0 : s0 + QB], in0=pp,
                        scalar=bq[:, g : g + 1], in1=qT8[:, g, 0, s0 : s0 + QB],
                        op0=mybir.AluOpType.add, op1=mybir.AluOpType.subtract)
                elif k_on_act:
                    nc.scalar.copy(kT8[:, g, s0 : s0 + QB], pp)
                else:
                    nc.vector.tensor_copy(out=kT8[:, g, s0 : s0 + QB], in_=pp)

        def proj_v(st):
            """V projection for s-tile st -> vA bf16 (fp8 DR, 3-term)."""
            pp = ps_m.tile([P, HD], F32, tag="m")
            for t in (0, 2, 4):
                nc.tensor.matmul(
                    pp, xh[:, t : t + 2, st * P : (st + 1) * P],
                    wslice("vh", 0, t, 2, 0, HD),
                    start=(t == 0), stop=False, perf_mode=DR)
            for i, t in enumerate((0, 2, 4)):
                nc.tensor.matmul(
                    pp, xls[:, t : t + 2, st * P : (st + 1) * P],
                    wslice("vhd", 0, t, 2, 0, HD),
                    start=False, stop=False, perf_mode=DR)
            for i, t in enumerate((0, 2, 4)):
                nc.tensor.matmul(
                    pp, xhd[:, t : t + 2, st * P : (st + 1) * P],
                    wslice("vls", 0, t, 2, 0, HD),
                    start=False, stop=(i == 2), perf_mode=DR)
            nc.vector.tensor_copy(
                out=vA[:, st, :, :], in_=pp.rearrange("p (n h) -> p n h", n=NHC))

        def scores_head(h, strip):
            """All score DRs + merged exps + masks for head h of strip."""
            g, e = divmod(h, 2)
            hp = e * H
            ets = ets0 if strip == 0 else ets1
            segs = SEG0 if strip == 0 else SEG1
            q0 = strip * QB
            groups = EXPG0 if strip == 0 else EXPG1
            for kts in groups:
                used = sum(segs[kt][1] for kt in kts)
                sm = ps_sm.tile([P, 2 * QB], F32, tag="sm")
                pcol = 0
                for kt in kts:
                    live = segs[kt][1]
                    o = (q0 + QB - live) - q0  # live q starts at q0 + o
                    kst = kT8[hp : hp + H, g, kt * P : (kt + 1) * P]
                    kst = kst.rearrange("p (o m) -> p o m", o=1).broadcast_to((H, 2, P))
                    nc.tensor.matmul(
                        sm[:, pcol : pcol + live], kst,
                        qT8[hp : hp + H, g, :, q0 + o : q0 + QB],
                        start=True, stop=True, perf_mode=DR)
                    pcol += live
                seg0 = segs[kts[0]][0]
                nc.scalar.activation(
                    ets[:, h, seg0 : seg0 + used], sm[:, 0:used],
                    mybir.ActivationFunctionType.Exp, scale=EXP_SCALE)
            # diagonal masks (ets *= tri on first 128 stored cols of diag kts)
            dkts = range(4) if strip == 0 else range(4, 8)
            for kt in dkts:
                so = segs[kt][0]
                nc.gpsimd.tensor_mul(
                    ets[:, h, so : so + P], ets[:, h, so : so + P], tri)

        def pv_qtile(strip, j):
            """z psum for q-tile j of strip: all heads' PV + denominators."""
            ets = ets0 if strip == 0 else ets1
            segs = SEG0 if strip == 0 else SEG1
            q0 = strip * QB
            zp = ps_z.tile([P, 390], F32, tag="z")
            gq = q0 + j * P  # global q block start
            for h in range(NHC):
                kts = [kt for kt in range(len(segs))
                       if kt * P <= gq + P - 1 and True]
                # live ktiles: those whose k-range start <= last q of block
                kts = [kt for kt in range(len(segs)) if kt * P < gq + P]
                for i, kt in enumerate(kts):
                    o = max(kt * P - q0, 0)
                    col = segs[kt][0] + (j * P - o)
                    st = ets[:, h, col : col + P]
                    nc.tensor.matmul(
                        zp[:, h * H : (h + 1) * H], st, vA[:, kt, h, :],
                        start=(i == 0), stop=(i == len(kts) - 1))
                for i, kt in enumerate(kts):
                    o = max(kt * P - q0, 0)
                    col = segs[kt][0] + (j * P - o)
                    st = ets[:, h, col : col + P]
                    nc.tensor.matmul(
                        zp[:, 384 + h : 385 + h], st, ones,
                        start=(i == 0), stop=(i == len(kts) - 1))
            return zp

        def phase3(strip, j, zp, last):
            """normalize -> transpose -> out-proj -> store for q-tile j."""
            row0 = strip * QB + j * P
            r = smalls.tile([P, NHC], F32)
            nc.vector.reciprocal(r, zp[:, 384:390])
            zn = znp.tile([P, NHC, H], BF)
            rb = r.rearrange("p (h o) -> p h o", o=1).broadcast_to((P, NHC, H))
            nc.vector.tensor_mul(
                zn, zp[:, 0:384].rearrange("p (n h) -> p n h", n=NHC), rb)
            znt_ps = ps_m.tile([P, NPAIR, P], BF, tag="m")
            for t in range(NPAIR):
                nc.tensor.matmul(
                    znt_ps[:, t, :], zn[:, 2 * t : 2 * t + 2, :].rearrange(
                        "p n h -> p (n h)"),
                    ident, is_transpose=True)
            znt = zntp.tile([P, NPAIR, P], BF)
            nc.vector.tensor_copy(out=znt, in_=znt_ps)
            osb = outp.tile([P, D], BF)
            for dh in range(2):
                op = ps_m.tile([P, D // 2], F32, tag="m")
                for t in range(NPAIR):
                    nc.tensor.matmul(
                        op, znt[:, t, :],
                        wo[:, t, dh * (D // 2) : (dh + 1) * (D // 2)],
                        start=(t == 0), stop=(t == NPAIR - 1))
                sl = osb[:, dh * (D // 2) : (dh + 1) * (D // 2)]
                if last and dh == 0:
                    nc.scalar.copy(sl, op)
                else:
                    nc.vector.tensor_copy(out=sl, in_=op)
                nc.sync.dma_start(
                    out=out_d[row0 : row0 + P, dh * (D // 2) : (dh + 1) * (D // 2)],
                    in_=sl)

        def phase3_tail(zpA, zpB):
            """Last two q-tiles: interleave DVE/Act chains to shorten the tail."""
            rows = (QB + 2 * P, QB + 3 * P)
            rA = smalls.tile([P, NHC], F32, name="rA")
            rB = smalls.tile([P, NHC], F32, name="rB")
            nc.vector.reciprocal(rA, zpA[:, 384:390])
            nc.vector.reciprocal(rB, zpB[:, 384:390])
            zns, znts = [], []
            for nm, zp, r in (("A", zpA, rA), ("B", zpB, rB)):
                zn = znp.tile([P, NHC, H], BF, name="znt_" + nm)
                rb = r.rearrange("p (h o) -> p h o", o=1).broadcast_to((P, NHC, H))
                nc.vector.tensor_mul(
                    zn, zp[:, 0:384].rearrange("p (n h) -> p n h", n=NHC), rb)
                zns.append(zn)
            for nm, zn in zip("AB", zns):
                znt_ps = ps_m.tile([P, NPAIR, P], BF, tag="m", name="znp_" + nm)
                for t in range(NPAIR):
                    nc.tensor.matmul(
                        znt_ps[:, t, :], zn[:, 2 * t : 2 * t + 2, :].rearrange(
                            "p n h -> p (n h)"),
                        ident, is_transpose=True)
                znt = zntp.tile([P, NPAIR, P], BF, name="zntt_" + nm)
                if nm == "A":
                    nc.scalar.copy(znt, znt_ps)
                else:
                    nc.vector.tensor_copy(out=znt, in_=znt_ps)
                znts.append(znt)
            for dh in range(2):
                for i, znt in enumerate(znts):
                    op = ps_m.tile([P, D // 2], F32, tag="m", name=f"op_{i}{dh}")
                    for t in range(NPAIR):
                        nc.tensor.matmul(
                            op, znt[:, t, :],
                            wo[:, t, dh * (D // 2) : (dh + 1) * (D // 2)],
                            start=(t == 0), stop=(t == NPAIR - 1))
                    osb = outp.tile([P, D // 2], BF, name=f"osb_{i}{dh}")
                    if (i + dh) % 2 == 0:
                        nc.scalar.copy(osb, op)
                    else:
                        nc.vector.tensor_copy(out=osb, in_=op)
                    nc.sync.dma_start(
                        out=out_d[rows[i] : rows[i] + P,
                                  dh * (D // 2) : (dh + 1) * (D // 2)],
                        in_=osb)

        # ================= schedule =================
        proj_qk(0, 0)
        scores_head(0, 0)
        scores_head(1, 0)
        proj_qk(1, 0)
        scores_head(2, 0)
        scores_head(3, 0)
        proj_qk(2, 0)
        scores_head(4, 0)
        scores_head(5, 0)
        proj_qk(0, 1)
        scores_head(0, 1)
        scores_head(1, 1)
        for st in range(4):
            proj_v(st)
        zp0 = pv_qtile(0, 0)
        proj_qk(1, 1)
        scores_head(2, 1)
        scores_head(3, 1)
        zp1 = pv_qtile(0, 1)
        phase3(0, 0, zp0, False)
        proj_qk(2, 1)
        scores_head(4, 1)
        scores_head(5, 1)
        zp2 = pv_qtile(0, 2)
        phase3(0, 1, zp1, False)
        for st in range(4, NST):
            proj_v(st)
        zp3 = pv_qtile(0, 3)
        phase3(0, 2, zp2, False)
        zp4 = pv_qtile(1, 0)
        phase3(0, 3, zp3, False)
        zp5 = pv_qtile(1, 1)
        phase3(1, 0, zp4, True)
        zp6 = pv_qtile(1, 2)
        phase3(1, 1, zp5, True)
        zp7 = pv_qtile(1, 3)
        phase3_tail(zp6, zp7)
        if DBG:
            nc.sync.dma_start(out=dq_d[:], in_=qT8.rearrange("p a b c -> p (a b c)"))
            nc.sync.dma_start(out=dk_d[:], in_=kT8.rearrange("p a b -> p (a b)"))
            nc.sync.dma_start(out=dets_d[:], in_=ets1.rearrange("p a b -> p (a b)"))
            nc.sync.dma_start(out=dva_d[:], in_=vA.rearrange("p a b c -> p (a b c)"))

    if not nc.is_finalized():
        nc.finalize()
    return nc


def _get_program():
    if "nc" not in _CACHE:
        _CACHE["nc"] = _build()
    return _CACHE["nc"]


F8NP = ml_dtypes.float8_e4m3
BFNP = ml_dtypes.bfloat16


def _q8(a):
    return a.astype(F8NP)


def make_in_maps(normalized_resid_pre, W_Q, W_K, W_V, b_Q):
    x = np.asarray(normalized_resid_pre, np.float32)
    W_Q = np.asarray(W_Q, np.float32) * FW
    W_K = np.asarray(W_K, np.float32) * FW
    W_V = np.asarray(W_V, np.float32) * FW
    b_Q = np.asarray(b_Q, np.float32) * FW

    tri = np.triu(np.ones((P, P), np.float32)).astype(BFNP)
    ident = np.eye(P, dtype=np.float32).astype(BFNP)
    trid = np.concatenate([tri, ident], axis=1)

    in_maps = []
    for c in range(8):
        b, hg = divmod(c, 2)
        hs = slice(hg * NHC, (hg + 1) * NHC)
        xt = np.ascontiguousarray(x[b].T)          # [D, S]
        xh = _q8(xt)
        xhf = xh.astype(np.float32)
        xls = _q8((xt - xhf) * 16.0)
        xhd = (xhf / 16.0).astype(F8NP)            # exact exponent shift

        wall = np.zeros((D, WALL_W), F8NP)
        for wname, W in (("q", W_Q), ("k", W_K), ("v", W_V)):
            Wg = W[hs].transpose(1, 0, 2).reshape(D, HD)  # [D, 6*64]
            Wh = _q8(Wg)
            Whf = Wh.astype(np.float32)
            Whd = (Whf / 16.0).astype(F8NP)
            Wls = _q8((Wg - Whf) * 16.0)
            if wname == "v":
                c0 = WALL_COLS["vh"][0]
                wall[:, c0 : c0 + HD] = Wh
                c0 = WALL_COLS["vhd"][0]
                wall[:, c0 : c0 + HD] = Whd
                c0 = WALL_COLS["vls"][0]
                wall[:, c0 : c0 + HD] = Wls
            else:
                for g in range(NPAIR):
                    sl = slice(g * P, (g + 1) * P)
                    c0 = WALL_COLS[wname + "h" + str(g)][0]
                    wall[:, c0 : c0 + P] = Wh[:, sl]
                    c0 = WALL_COLS[wname + "hd" + str(g)][0]
                    wall[:, c0 : c0 + P] = Whd[:, sl]
                    if QK_3T:
                        c0 = WALL_COLS[wname + "ls" + str(g)][0]
                        wall[:, c0 : c0 + P] = Wls[:, sl]

        in_maps.append({
            "wall": wall,
            "xh": xh,
            "xls": xls,
            "xhd": xhd,
            "wo": None,  # filled in kernel()
            "bq": np.ascontiguousarray(b_Q[hs].reshape(NPAIR, P).T),
            "trid": trid,
        })
    return in_maps


def kernel(
    normalized_resid_pre, W_Q, W_K, W_V, W_O, b_Q, b_K, b_V, b_O, **_unused
):
    W_O = np.asarray(W_O, np.float32)
    b_V, b_O = np.asarray(b_V, np.float32), np.asarray(b_O, np.float32)
    in_maps = make_in_maps(normalized_resid_pre, W_Q, W_K, W_V, b_Q)
    for c in range(8):
        hg = c % 2
        hs = slice(hg * NHC, (hg + 1) * NHC)
        in_maps[c]["wo"] = np.ascontiguousarray(
            (W_O[hs].reshape(HD, D) / FW).astype(BFNP))

    nc = _get_program()
    res = run_bass_kernel_spmd(nc, in_maps, list(range(8))).results

    out = np.zeros((B, S, D), np.float32)
    for c in range(8):
        out[c // 2] += res[c]["out"].astype(np.float32)
    out += b_O + np.einsum("nh,nhd->d", b_V, W_O)
    return out
